# revision 5
# baseline (speedup 1.0000x reference)
"""AttnAdaIN Trainium2 kernel (v2 — wire-optimized).

Computation (per batch b):
    F = f_w @ CK ; G = g_w @ SK ; Hh = h_w @ STY   (1x1 convs; biases folded
    or cancelled: per-query score offsets cancel in softmax)
    S = softmax_k(F^T G)          [HW, HW]
    mean = S @ Hh^T ; second = S @ (Hh^T)^2
    std = sqrt(relu(second - mean^2))
    out = std * mvn(content) + mean

End-to-end cost through the axon-proxied PJRT tunnel is dominated by
host<->device bytes (~40 MB/s pipe), so v2 minimizes wire traffic:
  * one batch per core on 4 cores — zero data duplication across cores
    (the 8-core query-split shipped sk/sty twice per batch);
  * everything ships as fp16 (PE matmuls run fp16 at the same 1 cycle/row
    as fp32r; the baseline already truncated operands to 11 mantissa bits);
  * per-channel content mean/var computed host-side (ships 2*C floats
    instead of requiring full content on every core);
  * all per-core inputs packed into ONE flat fp16 blob (one transfer, one
    concat) and the output returns as fp16.
On-chip flow matches the proven v1 flash kernel: scores from fp16
G''=W' SK and fp16 CK tiles; exp with fixed -30 shift on ScalarE into
f32r P (full exponent range keeps tiny probabilities); P·V / P·V^2 in
f32r with PSUM-resident accumulators; denominator via ones-vector matmul.
"""

import sys
import time

for _p in ("/opt/trn_rl_repo", "/opt/trn_rl_repo/concourse"):
    if _p not in sys.path:
        sys.path.insert(0, _p)

import contextlib

import numpy as np

import concourse.bacc as bacc
import concourse.mybir as mybir
import concourse.tile as tile

F32 = mybir.dt.float32
F32R = mybir.dt.float32r
F16 = mybir.dt.float16
AF = mybir.ActivationFunctionType
ALU = mybir.AluOpType

C = 512
HW = 4096
B = 4
N_CORES = 4
CC = C // 128
NK = HW // 128
Q_TILE = 256
NQ = HW // Q_TILE
NB = CC // 2  # psum banks per moment accumulator (2 c-chunks per bank)


def _blob_layout(with_score_bias, with_v_bias):
    segs = [("ck", C * HW), ("sk", C * HW), ("sty", C * HW), ("ct", C * HW),
            ("wT", C * C), ("hwT", C * C), ("stats", 128 * 2 * CC)]
    if with_score_bias:
        segs.append(("rbias", HW))
    if with_v_bias:
        segs.append(("hb", C))
    offs, off = {}, 0
    for name, n in segs:
        offs[name] = (off, n)
        off += n
    return offs, off


def build_program(with_score_bias=False, with_v_bias=False):
    offs, total = _blob_layout(with_score_bias, with_v_bias)

    nc = bacc.Bacc("TRN2", target_bir_lowering=False, debug=False,
                   num_devices=N_CORES)

    blob = nc.dram_tensor("blob", [total], F16, kind="ExternalInput")
    out = nc.dram_tensor("out", [C, HW], F16, kind="ExternalOutput")

    def seg3(name, p=128):
        o, n = offs[name]
        inner = n // (CC * p) if name not in ("wT", "hwT") else C
        return blob[o:o + n].rearrange("(c p q) -> c p q", c=CC, p=p, q=inner)

    ckr = seg3("ck")      # [CC, 128, HW]
    skr = seg3("sk")
    styr = seg3("sty")
    ctr = seg3("ct")
    wTr = seg3("wT")      # [CC, 128, C]
    hwTr = seg3("hwT")
    so, sn = offs["stats"]
    statr = blob[so:so + sn].rearrange("(p s) -> p s", p=128, s=2 * CC)
    if with_score_bias:
        ro, rn = offs["rbias"]
        rbias = blob[ro:ro + rn].rearrange("(one k) -> one k", one=1, k=HW)
    if with_v_bias:
        ho, hn = offs["hb"]
        hb = blob[ho:ho + hn].rearrange("(one c) -> one c", one=1, c=C)
    outr = out.rearrange("(c p) q -> c p q", p=128)

    with tile.TileContext(nc) as tc, contextlib.ExitStack() as ctx:
        persist = ctx.enter_context(tc.tile_pool(name="persist", bufs=1))
        ckpool = ctx.enter_context(tc.tile_pool(name="ckpool", bufs=2))
        ppool = ctx.enter_context(tc.tile_pool(name="ppool", bufs=4))
        epool = ctx.enter_context(tc.tile_pool(name="epool", bufs=2))
        opool = ctx.enter_context(tc.tile_pool(name="opool", bufs=2))
        ps_st = ctx.enter_context(
            tc.tile_pool(name="ps_st", bufs=3, space="PSUM"))
        ps_acc = ctx.enter_context(
            tc.tile_pool(name="ps_acc", bufs=1, space="PSUM"))
        ps_d = ctx.enter_context(
            tc.tile_pool(name="ps_d", bufs=1, space="PSUM"))
        dpool = ctx.enter_context(
            tc.tile_pool(name="dpool", bufs=2, space="DRAM"))

        # ---- constants (memset, nothing shipped) ----
        # memset only supports 32-bit value types; memset F32 then use a
        # bitcast view for the F32R/F16 matmul operands.
        ones_k_f = persist.tile([128, 1], F32, tag="ones_k")
        nc.vector.memset(ones_k_f, 1.0)
        ones_k = ones_k_f.bitcast(F32R)
        shift_sb = persist.tile([128, 1], F32, tag="shift")
        nc.vector.memset(shift_sb, -30.0)
        if with_score_bias or with_v_bias:
            ones_r_f = persist.tile([1, 64], F32, tag="ones_r")
            nc.vector.memset(ones_r_f, float(
                np.frombuffer(np.array([1.0, 1.0], np.float16).tobytes(),
                              np.float32)[0]))
            ones_r = ones_r_f.bitcast(F16)           # [1, 128] fp16 ones

        g2 = persist.tile([128, CC, HW], F16, tag="g2")
        vsb = persist.tile([128, NK, C], F32R, tag="v")
        v2sb = persist.tile([128, NK, C], F32R, tag="v2")
        mu = persist.tile([128, CC], F32, tag="mu")
        rstd = persist.tile([128, CC], F32, tag="rstd")
        if with_score_bias:
            r_sb = persist.tile([1, HW], F16, tag="rbias")
            nc.sync.dma_start(out=r_sb, in_=rbias[:])
        if with_v_bias:
            hb_sb = persist.tile([1, C], F16, tag="hb")
            nc.sync.dma_start(out=hb_sb, in_=hb[:])

        # ---- phase 0: weights, stats, G'' and V/V^2 precompute ----
        with tc.tile_pool(name="ph0", bufs=1) as ph0, \
             tc.tile_pool(name="ph0s", bufs=2) as ph0s:
            stat_sb = ph0.tile([128, 2 * CC], F16, tag="stats")
            nc.sync.dma_start(out=stat_sb, in_=statr[:])
            nc.scalar.copy(out=mu, in_=stat_sb[:, 0:CC])
            nc.scalar.copy(out=rstd, in_=stat_sb[:, CC:2 * CC])

            wT_sb = ph0.tile([128, CC, C], F16, tag="wT")
            hwT_sb = ph0.tile([128, CC, C], F16, tag="hwT")
            for c in range(CC):
                nc.sync.dma_start(out=wT_sb[:, c, :], in_=wTr[c])
                nc.sync.dma_start(out=hwT_sb[:, c, :], in_=hwTr[c])

            # G'' = W'^T SK  (score stationary operand), layout [c, k], fp16
            for ks in range(HW // 256):
                sl = slice(ks * 256, (ks + 1) * 256)
                sks = ph0s.tile([128, CC, 256], F16, tag="sk_stream")
                for b in range(CC):
                    nc.sync.dma_start(out=sks[:, b, :], in_=skr[b][:, sl])
                for a in range(CC):
                    gps = ps_st.tile([128, 256], F32, tag="st", name="gps")
                    for b in range(CC):
                        nc.tensor.matmul(
                            gps,
                            lhsT=wT_sb[:, b, a * 128:(a + 1) * 128],
                            rhs=sks[:, b, :],
                            start=(b == 0), stop=(b == CC - 1))
                    nc.scalar.copy(out=g2[:, a, sl], in_=gps)

            # V = STY^T hwT  ([k, c] in 128-row blocks) and V^2, f32r
            for kt in range(NK):
                sl = slice(kt * 128, (kt + 1) * 128)
                sts = ph0s.tile([128, CC, 128], F16, tag="sty_stream")
                for b in range(CC):
                    nc.sync.dma_start(out=sts[:, b, :], in_=styr[b][:, sl])
                vps = ps_st.tile([128, 512], F32, tag="st")
                for b in range(CC):
                    nc.tensor.matmul(vps[:, :C],
                                     lhsT=sts[:, b, :],
                                     rhs=hwT_sb[:, b, :],
                                     start=(b == 0), stop=(b == CC - 1))
                if with_v_bias:
                    nc.tensor.matmul(vps[:, :C],
                                     lhsT=ones_r,
                                     rhs=hb_sb,
                                     start=False, stop=True,
                                     skip_group_check=True)
                nc.scalar.copy(out=vsb[:, kt, :], in_=vps[:, :C])
                nc.vector.tensor_mul(v2sb[:, kt, :], vsb[:, kt, :],
                                     vsb[:, kt, :])

        # ---- flash main loop ----
        for qt in range(NQ):
            qsl = slice(qt * Q_TILE, (qt + 1) * Q_TILE)
            ckq = ckpool.tile([128, CC, Q_TILE], F16, tag="ckq")
            for c in range(CC):
                nc.sync.dma_start(out=ckq[:, c, :], in_=ckr[c][:, qsl])

            acc1 = [ps_acc.tile([128, 512], F32, tag=f"acc1_{i}",
                                name=f"acc1_{i}") for i in range(NB)]
            acc2 = [ps_acc.tile([128, 512], F32, tag=f"acc2_{i}",
                                name=f"acc2_{i}") for i in range(NB)]
            dps = ps_d.tile([1, Q_TILE], F32, tag="d")

            def acc_ap(accs, c):
                return accs[c // 2][:, (c % 2) * Q_TILE:(c % 2 + 1) * Q_TILE]

            # NOTE: start=True clears has_written bits for the WHOLE psum
            # bank, so each bank (2 c-chunks) forms a single accumulation
            # group: only its first matmul sets start.
            def emit_pv(kt, p):
                nc.tensor.matmul(dps, lhsT=ones_k, rhs=p,
                                 start=(kt == 0), stop=(kt == NK - 1),
                                 skip_group_check=True)
                for acc, lhs in ((acc1, vsb[:, kt, :]), (acc2, v2sb[:, kt, :])):
                    for c in range(CC):
                        csl = slice(c * 128, (c + 1) * 128)
                        nc.tensor.matmul(acc_ap(acc, c),
                                         lhsT=lhs[:, csl],
                                         rhs=p,
                                         start=(kt == 0 and c % 2 == 0),
                                         stop=(kt == NK - 1 and
                                               (c % 2 == 1 or c == CC - 1)),
                                         skip_group_check=True)

            # software pipeline: QK(kt) is emitted before PV(kt-1) so the PE
            # has score matmuls to run while ScalarE computes exp(kt-1).
            pending = []
            for kt in range(NK):
                ksl = slice(kt * 128, (kt + 1) * 128)
                st = ps_st.tile([128, Q_TILE], F32, tag="st")
                for c in range(CC):
                    nc.tensor.matmul(st,
                                     lhsT=g2[:, c, ksl],
                                     rhs=ckq[:, c, :],
                                     start=(c == 0),
                                     stop=(c == CC - 1 and not with_score_bias))
                if with_score_bias:
                    nc.tensor.matmul(st, lhsT=r_sb[:, ksl],
                                     rhs=ones_r[:, :Q_TILE],
                                     start=False, stop=True,
                                     skip_group_check=True)
                p = ppool.tile([128, Q_TILE], F32R, tag="p")
                nc.scalar.activation(out=p, in_=st, func=AF.Exp, bias=shift_sb)
                pending.append((kt, p))
                if len(pending) > 2:
                    emit_pv(*pending.pop(0))
            for item in pending:
                emit_pv(*item)

            # ---- epilogue for this q_tile ----
            rd = epool.tile([1, Q_TILE], F32, tag="rd", bufs=1)
            nc.vector.reciprocal(out=rd, in_=dps)
            rd_dram = dpool.tile([1, Q_TILE], F32, tag="rd_dram")
            nc.sync.dma_start(out=rd_dram, in_=rd)
            rdb = epool.tile([128, Q_TILE], F32, tag="rdb", bufs=1)
            nc.sync.dma_start(out=rdb,
                              in_=rd_dram.to_broadcast([128, Q_TILE]))

            avs, a2s = [], []
            for c in range(CC):
                av = epool.tile([128, Q_TILE], F32, tag=f"av{c}",
                                name=f"av{c}", bufs=1)
                nc.scalar.copy(out=av, in_=acc_ap(acc1, c))
                a2 = epool.tile([128, Q_TILE], F32, tag=f"a2{c}",
                                name=f"a2{c}", bufs=1)
                nc.scalar.copy(out=a2, in_=acc_ap(acc2, c))
                avs.append(av)
                a2s.append(a2)

            for c in range(CC):
                ctq = epool.tile([128, Q_TILE], F16, tag="ctq")
                nc.sync.dma_start(out=ctq, in_=ctr[c][:, qsl])
                ctf = epool.tile([128, Q_TILE], F32, tag="ctf", bufs=1)
                nc.scalar.copy(out=ctf, in_=ctq)
                mean = avs[c]
                nc.vector.tensor_mul(mean, avs[c], rdb)
                e2 = a2s[c]
                nc.vector.tensor_mul(e2, a2s[c], rdb)
                var = epool.tile([128, Q_TILE], F32, tag="var", bufs=1)
                nc.vector.tensor_mul(var, mean, mean)
                nc.vector.scalar_tensor_tensor(
                    out=var, in0=var, scalar=-1.0, in1=e2,
                    op0=ALU.mult, op1=ALU.add)
                nc.vector.tensor_scalar_max(var, var, 1e-38)
                std = var
                nc.scalar.activation(out=std, in_=var, func=AF.Ln)
                nc.scalar.activation(out=std, in_=std, func=AF.Exp, scale=0.5)
                normc = epool.tile([128, Q_TILE], F32, tag="normc", bufs=1)
                nc.vector.tensor_scalar(
                    out=normc, in0=ctf,
                    scalar1=mu[:, c:c + 1], scalar2=rstd[:, c:c + 1],
                    op0=ALU.subtract, op1=ALU.mult)
                o = opool.tile([128, Q_TILE], F32, tag="o")
                nc.vector.tensor_mul(o, std, normc)
                o16 = opool.tile([128, Q_TILE], F16, tag="o16")
                nc.vector.tensor_add(o16, o, mean)
                nc.sync.dma_start(out=outr[c][:, qsl], in_=o16)

    # Force exp/ln/copy onto the shared natural_log_exp_and_others table
    # set: the default per-function choice alternates exp_and_others <->
    # natural_log, costing ~2.7us per ACT_TABLE_LOAD, dozens of times.
    import concourse.bacc as bacc_mod
    _orig_tables = bacc_mod.get_activation_tables
    _keep = "natural_log_exp_and_others"
    _strip = {AF.Exp, AF.Ln, AF.Copy, AF.Identity}

    def _patched_tables(arch):
        t = _orig_tables(arch)
        for name, fns in t.items():
            if name != _keep:
                t[name] = fns - _strip
        return t

    bacc_mod.get_activation_tables = _patched_tables
    try:
        nc.compile()
    finally:
        bacc_mod.get_activation_tables = _orig_tables
    return nc


_PROGRAM_CACHE = {}
_EXEC_CACHE = {}


def _get_program(key):
    if key not in _PROGRAM_CACHE:
        with_r, with_hb = key
        _PROGRAM_CACHE[key] = build_program(
            with_score_bias=with_r, with_v_bias=with_hb)
    return _PROGRAM_CACHE[key]


def make_in_maps(content, style, content_key, style_key, f_w, f_b, g_w, g_b,
                 h_w, h_b):
    content = np.asarray(content, np.float32)
    with_r = bool(np.any(np.asarray(f_b)))
    with_hb = bool(np.any(np.asarray(h_b)))
    offs, total = _blob_layout(with_r, with_hb)

    ct32 = content.reshape(B, C, HW)
    mu = ct32.mean(axis=2)                                    # [B, C]
    var = ct32.var(axis=2, ddof=1) + 1e-5
    rstd = 1.0 / np.sqrt(var)

    f_w32 = np.asarray(f_w, np.float32)
    g_w32 = np.asarray(g_w, np.float32)
    wT = (g_w32.T @ f_w32).astype(np.float16)                 # [C, C]
    hwT = np.ascontiguousarray(np.asarray(h_w, np.float32).T).astype(
        np.float16)

    blob = np.empty((B, total), np.float16)

    def put(b, name, arr):
        o, n = offs[name]
        blob[b, o:o + n] = arr.reshape(-1)

    ck16 = np.asarray(content_key, np.float32).reshape(
        B, C, HW).astype(np.float16)
    sk16 = np.asarray(style_key, np.float32).reshape(
        B, C, HW).astype(np.float16)
    sty16 = np.asarray(style, np.float32).reshape(
        B, C, HW).astype(np.float16)
    ct16 = ct32.astype(np.float16)
    if with_r:
        u = g_w32.T.astype(np.float64) @ np.asarray(f_b, np.float64)   # [C]

    for b in range(B):
        put(b, "ck", ck16[b])
        put(b, "sk", sk16[b])
        put(b, "sty", sty16[b])
        put(b, "ct", ct16[b])
        put(b, "wT", wT)
        put(b, "hwT", hwT)
        # stats layout [128, 2*CC]: col c = mu chunk c, col CC+c = rstd chunk
        st = np.empty((128, 2 * CC), np.float16)
        for c in range(CC):
            st[:, c] = mu[b, c * 128:(c + 1) * 128]
            st[:, CC + c] = rstd[b, c * 128:(c + 1) * 128]
        put(b, "stats", st)
        if with_r:
            r = (u @ sk16[b].astype(np.float64)).astype(np.float16)
            put(b, "rbias", r)
        if with_hb:
            put(b, "hb", np.asarray(h_b, np.float16))

    in_maps = [{"blob": blob[b]} for b in range(B)]
    return in_maps, (with_r, with_hb)


def _get_exec(key):
    """Cached jitted shard_map executor over 4 cores (no per-call retrace)."""
    if key in _EXEC_CACHE:
        return _EXEC_CACHE[key]
    import jax
    import jax.numpy as jnp
    from jax.sharding import Mesh, PartitionSpec, NamedSharding
    try:
        from jax.experimental.shard_map import shard_map
    except ImportError:
        from jax import shard_map
    from concourse.bass2jax import (
        _bass_exec_p, install_neuronx_cc_hook, partition_id_tensor)

    install_neuronx_cc_hook()
    nc = _get_program(key)
    assert nc.dbg_addr is None
    pname = nc.partition_id_tensor.name if nc.partition_id_tensor else None

    in_names, out_names, out_avals = [], [], []
    for alloc in nc.m.functions[0].allocations:
        if not isinstance(alloc, mybir.MemoryLocationSet):
            continue
        name = alloc.memorylocations[0].name
        if alloc.kind == "ExternalInput":
            if name != pname:
                in_names.append(name)
        elif alloc.kind == "ExternalOutput":
            out_names.append(name)
            out_avals.append(jax.core.ShapedArray(
                tuple(alloc.tensor_shape), mybir.dt.np(alloc.dtype)))
    assert in_names == ["blob"] and out_names == ["out"]
    all_in_names = in_names + out_names
    if pname is not None:
        all_in_names.append(pname)

    def _body(*args):
        operands = list(args)
        if pname is not None:
            operands.append(partition_id_tensor())
        outs = _bass_exec_p.bind(
            *operands,
            out_avals=tuple(out_avals),
            in_names=tuple(all_in_names),
            out_names=tuple(out_names),
            lowering_input_output_aliases=(),
            sim_require_finite=True,
            sim_require_nnan=True,
            nc=nc,
        )
        return tuple(outs)

    devices = jax.devices()[:N_CORES]
    mesh = Mesh(np.asarray(devices), ("core",))
    spec = PartitionSpec("core")
    fn = jax.jit(
        shard_map(_body, mesh=mesh, in_specs=(spec, spec),
                  out_specs=(spec,), check_rep=False),
        donate_argnums=(1,),
        keep_unused=True,
    )
    sh = NamedSharding(mesh, spec)
    zshape = (N_CORES * C, HW)

    def zfn():
        return jax.jit(lambda: jnp.zeros(zshape, jnp.float16),
                       out_shardings=sh)()

    _EXEC_CACHE[key] = (fn, sh, zfn)
    return _EXEC_CACHE[key]


def kernel(**inputs):
    import jax
    in_maps, key = make_in_maps(**inputs)
    fn, sh, zfn = _get_exec(key)
    blob = np.ascontiguousarray(
        np.stack([m["blob"] for m in in_maps]).reshape(-1))
    xd = jax.device_put(blob, sh)
    zd = zfn()
    (out,) = fn(xd, zd)
    o = np.asarray(out)                    # [4*C, HW] fp16
    return o.reshape(B, C, 64, 64).astype(np.float32)


if __name__ == "__main__":
    rng = np.random.default_rng(0)
    inputs = {
        "content": rng.standard_normal((B, C, 64, 64)).astype(np.float32),
        "style": rng.standard_normal((B, C, 64, 64)).astype(np.float32),
        "content_key": rng.standard_normal((B, C, 64, 64)).astype(np.float32),
        "style_key": rng.standard_normal((B, C, 64, 64)).astype(np.float32),
        "f_w": (rng.standard_normal((C, C)) * 0.02).astype(np.float32),
        "f_b": np.zeros(C, np.float32),
        "g_w": (rng.standard_normal((C, C)) * 0.02).astype(np.float32),
        "g_b": np.zeros(C, np.float32),
        "h_w": (rng.standard_normal((C, C)) * 0.02).astype(np.float32),
        "h_b": np.zeros(C, np.float32),
    }
    t0 = time.time()
    out = kernel(**inputs)
    print("kernel done", out.shape, out.dtype, time.time() - t0)


# revision 7
# speedup vs baseline: 1.2064x; 1.2064x over previous
"""AttnAdaIN Trainium2 kernel (v2 — wire-optimized).

Computation (per batch b):
    F = f_w @ CK ; G = g_w @ SK ; Hh = h_w @ STY   (1x1 convs; biases folded
    or cancelled: per-query score offsets cancel in softmax)
    S = softmax_k(F^T G)          [HW, HW]
    mean = S @ Hh^T ; second = S @ (Hh^T)^2
    std = sqrt(relu(second - mean^2))
    out = std * mvn(content) + mean

End-to-end cost through the axon-proxied PJRT tunnel is dominated by
host<->device bytes (~40 MB/s pipe), so v2 minimizes wire traffic:
  * one batch per core on 4 cores — zero data duplication across cores
    (the 8-core query-split shipped sk/sty twice per batch);
  * everything ships as fp16 (PE matmuls run fp16 at the same 1 cycle/row
    as fp32r; the baseline already truncated operands to 11 mantissa bits);
  * per-channel content mean/var computed host-side (ships 2*C floats
    instead of requiring full content on every core);
  * all per-core inputs packed into ONE flat fp16 blob (one transfer, one
    concat) and the output returns as fp16.
On-chip flow matches the proven v1 flash kernel: scores from fp16
G''=W' SK and fp16 CK tiles; exp with fixed -30 shift on ScalarE into
f32r P (full exponent range keeps tiny probabilities); P·V / P·V^2 in
f32r with PSUM-resident accumulators; denominator via ones-vector matmul.
"""

import sys
import time

for _p in ("/opt/trn_rl_repo", "/opt/trn_rl_repo/concourse"):
    if _p not in sys.path:
        sys.path.insert(0, _p)

import contextlib

import numpy as np

import concourse.bacc as bacc
import concourse.mybir as mybir
import concourse.tile as tile

F32 = mybir.dt.float32
F32R = mybir.dt.float32r
F16 = mybir.dt.float16
AF = mybir.ActivationFunctionType
ALU = mybir.AluOpType

C = 512
HW = 4096
B = 4
N_CORES = 4
CC = C // 128
NK = HW // 128
Q_TILE = 256
NQ = HW // Q_TILE
NB = CC // 2  # psum banks per moment accumulator (2 c-chunks per bank)


def _blob_layout(with_score_bias, with_v_bias):
    segs = [("ck", C * HW), ("sk", C * HW), ("sty", C * HW), ("ct", C * HW),
            ("wT", C * C), ("hwT", C * C), ("stats", 128 * 2 * CC)]
    if with_score_bias:
        segs.append(("rbias", HW))
    if with_v_bias:
        segs.append(("hb", C))
    offs, off = {}, 0
    for name, n in segs:
        offs[name] = (off, n)
        off += n
    return offs, off


def build_program(with_score_bias=False, with_v_bias=False):
    offs, total = _blob_layout(with_score_bias, with_v_bias)

    nc = bacc.Bacc("TRN2", target_bir_lowering=False, debug=False,
                   num_devices=N_CORES)

    blob = nc.dram_tensor("blob", [total], F16, kind="ExternalInput")
    out = nc.dram_tensor("out", [C, HW], F16, kind="ExternalOutput")

    def seg3(name, p=128):
        o, n = offs[name]
        inner = n // (CC * p) if name not in ("wT", "hwT") else C
        return blob[o:o + n].rearrange("(c p q) -> c p q", c=CC, p=p, q=inner)

    ckr = seg3("ck")      # [CC, 128, HW]
    skr = seg3("sk")
    styr = seg3("sty")
    ctr = seg3("ct")
    wTr = seg3("wT")      # [CC, 128, C]
    hwTr = seg3("hwT")
    so, sn = offs["stats"]
    statr = blob[so:so + sn].rearrange("(p s) -> p s", p=128, s=2 * CC)
    if with_score_bias:
        ro, rn = offs["rbias"]
        rbias = blob[ro:ro + rn].rearrange("(one k) -> one k", one=1, k=HW)
    if with_v_bias:
        ho, hn = offs["hb"]
        hb = blob[ho:ho + hn].rearrange("(one c) -> one c", one=1, c=C)
    outr = out.rearrange("(c p) q -> c p q", p=128)

    with tile.TileContext(nc) as tc, contextlib.ExitStack() as ctx:
        persist = ctx.enter_context(tc.tile_pool(name="persist", bufs=1))
        ckpool = ctx.enter_context(tc.tile_pool(name="ckpool", bufs=2))
        ppool = ctx.enter_context(tc.tile_pool(name="ppool", bufs=4))
        epool = ctx.enter_context(tc.tile_pool(name="epool", bufs=2))
        opool = ctx.enter_context(tc.tile_pool(name="opool", bufs=2))
        ps_st = ctx.enter_context(
            tc.tile_pool(name="ps_st", bufs=3, space="PSUM"))
        ps_acc = ctx.enter_context(
            tc.tile_pool(name="ps_acc", bufs=1, space="PSUM"))
        ps_d = ctx.enter_context(
            tc.tile_pool(name="ps_d", bufs=1, space="PSUM"))
        dpool = ctx.enter_context(
            tc.tile_pool(name="dpool", bufs=2, space="DRAM"))

        # ---- constants (memset, nothing shipped) ----
        # memset only supports 32-bit value types; memset F32 then use a
        # bitcast view for the F32R/F16 matmul operands.
        ones_k_f = persist.tile([128, 1], F32, tag="ones_k")
        nc.vector.memset(ones_k_f, 1.0)
        ones_k = ones_k_f.bitcast(F32R)
        shift_sb = persist.tile([128, 1], F32, tag="shift")
        nc.vector.memset(shift_sb, -30.0)
        if with_score_bias or with_v_bias:
            ones_r_f = persist.tile([1, 64], F32, tag="ones_r")
            nc.vector.memset(ones_r_f, float(
                np.frombuffer(np.array([1.0, 1.0], np.float16).tobytes(),
                              np.float32)[0]))
            ones_r = ones_r_f.bitcast(F16)           # [1, 128] fp16 ones

        g2 = persist.tile([128, CC, HW], F16, tag="g2")
        vsb = persist.tile([128, NK, C], F32R, tag="v")
        v2sb = persist.tile([128, NK, C], F32R, tag="v2")
        mu = persist.tile([128, CC], F32, tag="mu")
        rstd = persist.tile([128, CC], F32, tag="rstd")
        if with_score_bias:
            r_sb = persist.tile([1, HW], F16, tag="rbias")
            nc.sync.dma_start(out=r_sb, in_=rbias[:])
        if with_v_bias:
            hb_sb = persist.tile([1, C], F16, tag="hb")
            nc.sync.dma_start(out=hb_sb, in_=hb[:])

        # ---- phase 0: weights, stats, G'' and V/V^2 precompute ----
        with tc.tile_pool(name="ph0", bufs=1) as ph0, \
             tc.tile_pool(name="ph0s", bufs=2) as ph0s:
            stat_sb = ph0.tile([128, 2 * CC], F16, tag="stats")
            nc.sync.dma_start(out=stat_sb, in_=statr[:])
            nc.scalar.copy(out=mu, in_=stat_sb[:, 0:CC])
            nc.scalar.copy(out=rstd, in_=stat_sb[:, CC:2 * CC])

            wT_sb = ph0.tile([128, CC, C], F16, tag="wT")
            hwT_sb = ph0.tile([128, CC, C], F16, tag="hwT")
            for c in range(CC):
                nc.sync.dma_start(out=wT_sb[:, c, :], in_=wTr[c])
                nc.sync.dma_start(out=hwT_sb[:, c, :], in_=hwTr[c])

            # G'' = W'^T SK  (score stationary operand), layout [c, k], fp16
            for ks in range(HW // 256):
                sl = slice(ks * 256, (ks + 1) * 256)
                sks = ph0s.tile([128, CC, 256], F16, tag="sk_stream")
                for b in range(CC):
                    nc.sync.dma_start(out=sks[:, b, :], in_=skr[b][:, sl])
                for a in range(CC):
                    gps = ps_st.tile([128, 256], F32, tag="st", name="gps")
                    for b in range(CC):
                        nc.tensor.matmul(
                            gps,
                            lhsT=wT_sb[:, b, a * 128:(a + 1) * 128],
                            rhs=sks[:, b, :],
                            start=(b == 0), stop=(b == CC - 1))
                    nc.scalar.copy(out=g2[:, a, sl], in_=gps)

            # V = STY^T hwT  ([k, c] in 128-row blocks) and V^2, f32r
            for kt in range(NK):
                sl = slice(kt * 128, (kt + 1) * 128)
                sts = ph0s.tile([128, CC, 128], F16, tag="sty_stream")
                for b in range(CC):
                    nc.sync.dma_start(out=sts[:, b, :], in_=styr[b][:, sl])
                vps = ps_st.tile([128, 512], F32, tag="st")
                for b in range(CC):
                    nc.tensor.matmul(vps[:, :C],
                                     lhsT=sts[:, b, :],
                                     rhs=hwT_sb[:, b, :],
                                     start=(b == 0), stop=(b == CC - 1))
                if with_v_bias:
                    nc.tensor.matmul(vps[:, :C],
                                     lhsT=ones_r,
                                     rhs=hb_sb,
                                     start=False, stop=True,
                                     skip_group_check=True)
                nc.scalar.copy(out=vsb[:, kt, :], in_=vps[:, :C])
                nc.vector.tensor_mul(v2sb[:, kt, :], vsb[:, kt, :],
                                     vsb[:, kt, :])

        # ---- flash main loop ----
        for qt in range(NQ):
            qsl = slice(qt * Q_TILE, (qt + 1) * Q_TILE)
            ckq = ckpool.tile([128, CC, Q_TILE], F16, tag="ckq")
            for c in range(CC):
                nc.sync.dma_start(out=ckq[:, c, :], in_=ckr[c][:, qsl])

            acc1 = [ps_acc.tile([128, 512], F32, tag=f"acc1_{i}",
                                name=f"acc1_{i}") for i in range(NB)]
            acc2 = [ps_acc.tile([128, 512], F32, tag=f"acc2_{i}",
                                name=f"acc2_{i}") for i in range(NB)]
            dps = ps_d.tile([1, Q_TILE], F32, tag="d")

            def acc_ap(accs, c):
                return accs[c // 2][:, (c % 2) * Q_TILE:(c % 2 + 1) * Q_TILE]

            # NOTE: start=True clears has_written bits for the WHOLE psum
            # bank, so each bank (2 c-chunks) forms a single accumulation
            # group: only its first matmul sets start.
            def emit_pv(kt, p):
                nc.tensor.matmul(dps, lhsT=ones_k, rhs=p,
                                 start=(kt == 0), stop=(kt == NK - 1),
                                 skip_group_check=True)
                for acc, lhs in ((acc1, vsb[:, kt, :]), (acc2, v2sb[:, kt, :])):
                    for c in range(CC):
                        csl = slice(c * 128, (c + 1) * 128)
                        nc.tensor.matmul(acc_ap(acc, c),
                                         lhsT=lhs[:, csl],
                                         rhs=p,
                                         start=(kt == 0 and c % 2 == 0),
                                         stop=(kt == NK - 1 and
                                               (c % 2 == 1 or c == CC - 1)),
                                         skip_group_check=True)

            # software pipeline: QK(kt) is emitted before PV(kt-1) so the PE
            # has score matmuls to run while ScalarE computes exp(kt-1).
            pending = []
            for kt in range(NK):
                ksl = slice(kt * 128, (kt + 1) * 128)
                st = ps_st.tile([128, Q_TILE], F32, tag="st")
                for c in range(CC):
                    nc.tensor.matmul(st,
                                     lhsT=g2[:, c, ksl],
                                     rhs=ckq[:, c, :],
                                     start=(c == 0),
                                     stop=(c == CC - 1 and not with_score_bias))
                if with_score_bias:
                    nc.tensor.matmul(st, lhsT=r_sb[:, ksl],
                                     rhs=ones_r[:, :Q_TILE],
                                     start=False, stop=True,
                                     skip_group_check=True)
                p = ppool.tile([128, Q_TILE], F32R, tag="p")
                nc.scalar.activation(out=p, in_=st, func=AF.Exp, bias=shift_sb)
                pending.append((kt, p))
                if len(pending) > 2:
                    emit_pv(*pending.pop(0))
            for item in pending:
                emit_pv(*item)

            # ---- epilogue for this q_tile ----
            rd = epool.tile([1, Q_TILE], F32, tag="rd", bufs=1)
            nc.vector.reciprocal(out=rd, in_=dps)
            rd_dram = dpool.tile([1, Q_TILE], F32, tag="rd_dram")
            nc.sync.dma_start(out=rd_dram, in_=rd)
            rdb = epool.tile([128, Q_TILE], F32, tag="rdb", bufs=1)
            nc.sync.dma_start(out=rdb,
                              in_=rd_dram.to_broadcast([128, Q_TILE]))

            avs, a2s = [], []
            for c in range(CC):
                av = epool.tile([128, Q_TILE], F32, tag=f"av{c}",
                                name=f"av{c}", bufs=1)
                nc.scalar.copy(out=av, in_=acc_ap(acc1, c))
                a2 = epool.tile([128, Q_TILE], F32, tag=f"a2{c}",
                                name=f"a2{c}", bufs=1)
                nc.scalar.copy(out=a2, in_=acc_ap(acc2, c))
                avs.append(av)
                a2s.append(a2)

            for c in range(CC):
                ctq = epool.tile([128, Q_TILE], F16, tag="ctq")
                nc.sync.dma_start(out=ctq, in_=ctr[c][:, qsl])
                ctf = epool.tile([128, Q_TILE], F32, tag="ctf", bufs=1)
                nc.scalar.copy(out=ctf, in_=ctq)
                mean = avs[c]
                nc.vector.tensor_mul(mean, avs[c], rdb)
                e2 = a2s[c]
                nc.vector.tensor_mul(e2, a2s[c], rdb)
                var = epool.tile([128, Q_TILE], F32, tag="var", bufs=1)
                nc.vector.tensor_mul(var, mean, mean)
                nc.vector.scalar_tensor_tensor(
                    out=var, in0=var, scalar=-1.0, in1=e2,
                    op0=ALU.mult, op1=ALU.add)
                nc.vector.tensor_scalar_max(var, var, 1e-38)
                std = var
                nc.scalar.activation(out=std, in_=var, func=AF.Ln)
                nc.scalar.activation(out=std, in_=std, func=AF.Exp, scale=0.5)
                normc = epool.tile([128, Q_TILE], F32, tag="normc", bufs=1)
                nc.vector.tensor_scalar(
                    out=normc, in0=ctf,
                    scalar1=mu[:, c:c + 1], scalar2=rstd[:, c:c + 1],
                    op0=ALU.subtract, op1=ALU.mult)
                o = opool.tile([128, Q_TILE], F32, tag="o")
                nc.vector.tensor_mul(o, std, normc)
                o16 = opool.tile([128, Q_TILE], F16, tag="o16")
                nc.vector.tensor_add(o16, o, mean)
                nc.sync.dma_start(out=outr[c][:, qsl], in_=o16)

    # Force exp/ln/copy onto the shared natural_log_exp_and_others table
    # set: the default per-function choice alternates exp_and_others <->
    # natural_log, costing ~2.7us per ACT_TABLE_LOAD, dozens of times.
    import concourse.bacc as bacc_mod
    _orig_tables = bacc_mod.get_activation_tables
    _keep = "natural_log_exp_and_others"
    _strip = {AF.Exp, AF.Ln, AF.Copy, AF.Identity}

    def _patched_tables(arch):
        t = _orig_tables(arch)
        for name, fns in t.items():
            if name != _keep:
                t[name] = fns - _strip
        return t

    bacc_mod.get_activation_tables = _patched_tables
    try:
        nc.compile()
    finally:
        bacc_mod.get_activation_tables = _orig_tables
    return nc


_PROGRAM_CACHE = {}
_EXEC_CACHE = {}


def _get_program(key):
    if key not in _PROGRAM_CACHE:
        with_r, with_hb = key
        _PROGRAM_CACHE[key] = build_program(
            with_score_bias=with_r, with_v_bias=with_hb)
    return _PROGRAM_CACHE[key]


def prep_inputs(content, style, content_key, style_key, f_w, f_b, g_w, g_b,
                h_w, h_b):
    with_r = bool(np.any(np.asarray(f_b)))
    with_hb = bool(np.any(np.asarray(h_b)))
    key = (with_r, with_hb)

    ct32 = np.asarray(content, np.float32).reshape(B, C, HW)
    ck32 = np.asarray(content_key, np.float32).reshape(B, C, HW)
    sk32 = np.asarray(style_key, np.float32).reshape(B, C, HW)
    sty32 = np.asarray(style, np.float32).reshape(B, C, HW)

    f_w32 = np.asarray(f_w, np.float32)
    g_w32 = np.asarray(g_w, np.float32)
    wT16 = (g_w32.T @ f_w32).astype(np.float16)               # [C, C]
    hwT16 = np.ascontiguousarray(np.asarray(h_w, np.float32).T).astype(
        np.float16)
    u = (g_w32.T.astype(np.float64) @ np.asarray(f_b, np.float64)
         if with_r else None)
    hb16 = np.asarray(h_b, np.float16) if with_hb else None
    return key, (key, ct32, ck32, sk32, sty32, wT16, hwT16, u, hb16)


def make_blob(b, key, ct32, ck32, sk32, sty32, wT16, hwT16, u, hb16):
    with_r, with_hb = key
    offs, total = _blob_layout(with_r, with_hb)
    blob = np.empty(total, np.float16)

    def put(name, arr):
        o, n = offs[name]
        blob[o:o + n] = arr.reshape(-1)       # cast to fp16 on assignment

    put("ck", ck32[b])
    put("sk", sk32[b])
    put("sty", sty32[b])
    put("ct", ct32[b])
    put("wT", wT16)
    put("hwT", hwT16)
    ctb = ct32[b]
    mu = ctb.mean(axis=1)                                      # [C]
    var = ctb.var(axis=1, ddof=1) + 1e-5
    rstd = 1.0 / np.sqrt(var)
    # stats layout [128, 2*CC]: col c = mu chunk c, col CC+c = rstd chunk
    st = np.empty((128, 2 * CC), np.float16)
    for c in range(CC):
        st[:, c] = mu[c * 128:(c + 1) * 128]
        st[:, CC + c] = rstd[c * 128:(c + 1) * 128]
    put("stats", st)
    if with_r:
        put("rbias", (u @ sk32[b].astype(np.float64)).astype(np.float16))
    if with_hb:
        put("hb", hb16)
    return blob


def make_in_maps(content, style, content_key, style_key, f_w, f_b, g_w, g_b,
                 h_w, h_b):
    key, prep = prep_inputs(content, style, content_key, style_key, f_w,
                            f_b, g_w, g_b, h_w, h_b)
    in_maps = [{"blob": make_blob(b, *prep)} for b in range(B)]
    return in_maps, key


def _get_exec(key):
    """Cached per-device jitted executors (one per core, no retrace).

    Per-core dispatch (instead of one shard_map call) lets the output
    fetch of core i overlap the input upload of core i+1 — the axon pipe
    is full-duplex, so the D2H leg hides entirely under the H2D stream.
    """
    if key in _EXEC_CACHE:
        return _EXEC_CACHE[key]
    import jax
    import jax.numpy as jnp
    from concourse.bass2jax import (
        _bass_exec_p, install_neuronx_cc_hook, partition_id_tensor)

    install_neuronx_cc_hook()
    nc = _get_program(key)
    assert nc.dbg_addr is None
    pname = nc.partition_id_tensor.name if nc.partition_id_tensor else None

    in_names, out_names, out_avals = [], [], []
    for alloc in nc.m.functions[0].allocations:
        if not isinstance(alloc, mybir.MemoryLocationSet):
            continue
        name = alloc.memorylocations[0].name
        if alloc.kind == "ExternalInput":
            if name != pname:
                in_names.append(name)
        elif alloc.kind == "ExternalOutput":
            out_names.append(name)
            out_avals.append(jax.core.ShapedArray(
                tuple(alloc.tensor_shape), mybir.dt.np(alloc.dtype)))
    assert in_names == ["blob"] and out_names == ["out"]
    all_in_names = in_names + out_names
    if pname is not None:
        all_in_names.append(pname)

    def _body(*args):
        operands = list(args)
        if pname is not None:
            operands.append(partition_id_tensor())
        outs = _bass_exec_p.bind(
            *operands,
            out_avals=tuple(out_avals),
            in_names=tuple(all_in_names),
            out_names=tuple(out_names),
            lowering_input_output_aliases=(),
            sim_require_finite=True,
            sim_require_nnan=True,
            nc=nc,
        )
        return tuple(outs)

    devices = jax.devices()[:N_CORES]
    fns = [jax.jit(_body, donate_argnums=(1,), keep_unused=True)
           for _ in devices]
    zfns = [jax.jit(lambda: jnp.zeros((C, HW), jnp.float16), device=d)
            for d in devices]
    _EXEC_CACHE[key] = (fns, zfns, devices)
    return _EXEC_CACHE[key]


def kernel(**inputs):
    import jax
    key, prep = prep_inputs(**inputs)
    fns, zfns, devices = _get_exec(key)
    outs = []
    for b in range(B):
        blob = make_blob(b, *prep)             # host prep overlaps transfers
        xd = jax.device_put(blob, devices[b])
        (o,) = fns[b](xd, zfns[b]())
        o.copy_to_host_async()
        outs.append(o)
    res = np.empty((B, C, HW), np.float32)
    for b in range(B):
        res[b] = np.asarray(outs[b])           # fetch rides under later H2D
    return res.reshape(B, C, 64, 64)


if __name__ == "__main__":
    rng = np.random.default_rng(0)
    inputs = {
        "content": rng.standard_normal((B, C, 64, 64)).astype(np.float32),
        "style": rng.standard_normal((B, C, 64, 64)).astype(np.float32),
        "content_key": rng.standard_normal((B, C, 64, 64)).astype(np.float32),
        "style_key": rng.standard_normal((B, C, 64, 64)).astype(np.float32),
        "f_w": (rng.standard_normal((C, C)) * 0.02).astype(np.float32),
        "f_b": np.zeros(C, np.float32),
        "g_w": (rng.standard_normal((C, C)) * 0.02).astype(np.float32),
        "g_b": np.zeros(C, np.float32),
        "h_w": (rng.standard_normal((C, C)) * 0.02).astype(np.float32),
        "h_b": np.zeros(C, np.float32),
    }
    t0 = time.time()
    out = kernel(**inputs)
    print("kernel done", out.shape, out.dtype, time.time() - t0)


# revision 12
# speedup vs baseline: 1.4754x; 1.2230x over previous
"""AttnAdaIN Trainium2 kernel (v2 — wire-optimized).

Computation (per batch b):
    F = f_w @ CK ; G = g_w @ SK ; Hh = h_w @ STY   (1x1 convs; biases folded
    or cancelled: per-query score offsets cancel in softmax)
    S = softmax_k(F^T G)          [HW, HW]
    mean = S @ Hh^T ; second = S @ (Hh^T)^2
    std = sqrt(relu(second - mean^2))
    out = std * mvn(content) + mean

End-to-end cost through the axon-proxied PJRT tunnel is dominated by
host<->device bytes (~40 MB/s pipe), so v2 minimizes wire traffic:
  * one batch per core on 4 cores — zero data duplication across cores
    (the 8-core query-split shipped sk/sty twice per batch);
  * everything ships as fp16 (PE matmuls run fp16 at the same 1 cycle/row
    as fp32r; the baseline already truncated operands to 11 mantissa bits);
  * per-channel content mean/var computed host-side (ships 2*C floats
    instead of requiring full content on every core);
  * all per-core inputs packed into ONE flat fp16 blob (one transfer, one
    concat) and the output returns as fp16.
On-chip flow matches the proven v1 flash kernel: scores from fp16
G''=W' SK and fp16 CK tiles; exp with fixed -30 shift on ScalarE into
f32r P (full exponent range keeps tiny probabilities); P·V / P·V^2 in
f32r with PSUM-resident accumulators; denominator via ones-vector matmul.
"""

import sys
import time

for _p in ("/opt/trn_rl_repo", "/opt/trn_rl_repo/concourse"):
    if _p not in sys.path:
        sys.path.insert(0, _p)

import contextlib

import numpy as np

import concourse.bacc as bacc
import concourse.mybir as mybir
import concourse.tile as tile

F32 = mybir.dt.float32
F32R = mybir.dt.float32r
F16 = mybir.dt.float16
AF = mybir.ActivationFunctionType
ALU = mybir.AluOpType

C = 512
HW = 4096
B = 4
N_CORES = 4
CC = C // 128
NK = HW // 128
Q_TILE = 256
NQ = HW // Q_TILE
NB = CC // 2  # psum banks per moment accumulator (2 c-chunks per bank)


def _blob_layout(with_score_bias, with_v_bias):
    """Byte-addressed segments of the per-core uint8 input blob.

    fp16 segments first (keeps 2-byte alignment), int8 last. Values are
    (byte_offset, elem_count, itemsize).
    """
    segs = [("ck", C * HW, 2), ("sk", C * HW, 2), ("wT", C * C, 2),
            ("hwT", C * C, 2), ("stats", 128 * 2 * CC, 2)]
    if with_score_bias:
        segs.append(("rbias", HW, 2))
    if with_v_bias:
        segs.append(("hb", C, 2))
    segs += [("sty", C * HW, 1), ("ct", C * HW, 1)]
    offs, off = {}, 0
    for name, n, isz in segs:
        offs[name] = (off, n, isz)
        off += n * isz
    return offs, off


def build_program(with_score_bias=False, with_v_bias=False):
    offs, total = _blob_layout(with_score_bias, with_v_bias)

    nc = bacc.Bacc("TRN2", target_bir_lowering=False, debug=False,
                   num_devices=N_CORES)

    I8 = mybir.dt.int8
    U8 = mybir.dt.uint8
    blob = nc.dram_tensor("blob", [total], U8, kind="ExternalInput")
    out = nc.dram_tensor("out", [C, HW], F16, kind="ExternalOutput")

    def seg(name, dt):
        o, n, isz = offs[name]
        return blob[o:o + n * isz].bitcast(dt)

    def seg3(name, dt=F16):
        v = seg(name, dt)
        inner = v.shape[0] // (CC * 128)
        return v.rearrange("(c p q) -> c p q", c=CC, p=128, q=inner)

    ckr = seg3("ck")            # [CC, 128, HW] fp16
    skr = seg3("sk")
    styr = seg3("sty", I8)      # [CC, 128, HW] int8
    ctr = seg3("ct", I8)
    wTr = seg3("wT")            # [CC, 128, C] fp16
    hwTr = seg3("hwT")
    statr = seg("stats", F16).rearrange("(p s) -> p s", p=128, s=2 * CC)
    if with_score_bias:
        rbias = seg("rbias", F16).rearrange("(one k) -> one k", one=1, k=HW)
    if with_v_bias:
        hb = seg("hb", F16).rearrange("(one c) -> one c", one=1, c=C)
    outr = out.rearrange("(c p) q -> c p q", p=128)

    with tile.TileContext(nc) as tc, contextlib.ExitStack() as ctx:
        persist = ctx.enter_context(tc.tile_pool(name="persist", bufs=1))
        ckpool = ctx.enter_context(tc.tile_pool(name="ckpool", bufs=2))
        ppool = ctx.enter_context(tc.tile_pool(name="ppool", bufs=4))
        epool = ctx.enter_context(tc.tile_pool(name="epool", bufs=2))
        opool = ctx.enter_context(tc.tile_pool(name="opool", bufs=2))
        ps_st = ctx.enter_context(
            tc.tile_pool(name="ps_st", bufs=3, space="PSUM"))
        ps_acc = ctx.enter_context(
            tc.tile_pool(name="ps_acc", bufs=1, space="PSUM"))
        ps_d = ctx.enter_context(
            tc.tile_pool(name="ps_d", bufs=1, space="PSUM"))
        dpool = ctx.enter_context(
            tc.tile_pool(name="dpool", bufs=2, space="DRAM"))

        # ---- constants (memset, nothing shipped) ----
        # memset only supports 32-bit value types; memset F32 then use a
        # bitcast view for the F32R/F16 matmul operands.
        ones_k_f = persist.tile([128, 1], F32, tag="ones_k")
        nc.vector.memset(ones_k_f, 1.0)
        ones_k = ones_k_f.bitcast(F32R)
        shift_sb = persist.tile([128, 1], F32, tag="shift")
        nc.vector.memset(shift_sb, -30.0)
        if with_score_bias or with_v_bias:
            ones_r_f = persist.tile([1, 64], F32, tag="ones_r")
            nc.vector.memset(ones_r_f, float(
                np.frombuffer(np.array([1.0, 1.0], np.float16).tobytes(),
                              np.float32)[0]))
            ones_r = ones_r_f.bitcast(F16)           # [1, 128] fp16 ones

        g2 = persist.tile([128, CC, HW], F16, tag="g2")
        vsb = persist.tile([128, NK, C], F32R, tag="v")
        v2sb = persist.tile([128, NK, C], F32R, tag="v2")
        mu = persist.tile([128, CC], F32, tag="mu")
        rstd = persist.tile([128, CC], F32, tag="rstd")
        if with_score_bias:
            r_sb = persist.tile([1, HW], F16, tag="rbias")
            nc.sync.dma_start(out=r_sb, in_=rbias[:])
        if with_v_bias:
            hb_sb = persist.tile([1, C], F16, tag="hb")
            nc.sync.dma_start(out=hb_sb, in_=hb[:])

        # ---- phase 0: weights, stats, G'' and V/V^2 precompute ----
        with tc.tile_pool(name="ph0", bufs=1) as ph0, \
             tc.tile_pool(name="ph0s", bufs=2) as ph0s:
            stat_sb = ph0.tile([128, 2 * CC], F16, tag="stats")
            nc.sync.dma_start(out=stat_sb, in_=statr[:])
            nc.scalar.copy(out=mu, in_=stat_sb[:, 0:CC])
            nc.scalar.copy(out=rstd, in_=stat_sb[:, CC:2 * CC])

            wT_sb = ph0.tile([128, CC, C], F16, tag="wT")
            hwT_sb = ph0.tile([128, CC, C], F16, tag="hwT")
            for c in range(CC):
                nc.sync.dma_start(out=wT_sb[:, c, :], in_=wTr[c])
                nc.sync.dma_start(out=hwT_sb[:, c, :], in_=hwTr[c])

            # G'' = W'^T SK  (score stationary operand), layout [c, k], fp16
            for ks in range(HW // 256):
                sl = slice(ks * 256, (ks + 1) * 256)
                sks = ph0s.tile([128, CC, 256], F16, tag="sk_stream")
                for b in range(CC):
                    nc.sync.dma_start(out=sks[:, b, :], in_=skr[b][:, sl])
                for a in range(CC):
                    gps = ps_st.tile([128, 256], F32, tag="st", name="gps")
                    for b in range(CC):
                        nc.tensor.matmul(
                            gps,
                            lhsT=wT_sb[:, b, a * 128:(a + 1) * 128],
                            rhs=sks[:, b, :],
                            start=(b == 0), stop=(b == CC - 1))
                    nc.scalar.copy(out=g2[:, a, sl], in_=gps)

            # V = STY^T hwT  ([k, c] in 128-row blocks) and V^2, f32r.
            # STY ships int8 (per-channel scales folded into hwT host-side);
            # cast int8 -> fp16 on ScalarE before the matmul.
            for kt in range(NK):
                sl = slice(kt * 128, (kt + 1) * 128)
                sts8 = ph0s.tile([128, CC, 128], mybir.dt.int8,
                                 tag="sty_stream8")
                for b in range(CC):
                    nc.sync.dma_start(out=sts8[:, b, :], in_=styr[b][:, sl])
                sts = ph0s.tile([128, CC, 128], F16, tag="sty_stream")
                nc.scalar.copy(out=sts[:, :, :], in_=sts8[:, :, :])
                vps = ps_st.tile([128, 512], F32, tag="st")
                for b in range(CC):
                    nc.tensor.matmul(vps[:, :C],
                                     lhsT=sts[:, b, :],
                                     rhs=hwT_sb[:, b, :],
                                     start=(b == 0), stop=(b == CC - 1))
                if with_v_bias:
                    nc.tensor.matmul(vps[:, :C],
                                     lhsT=ones_r,
                                     rhs=hb_sb,
                                     start=False, stop=True,
                                     skip_group_check=True)
                nc.scalar.copy(out=vsb[:, kt, :], in_=vps[:, :C])
                nc.vector.tensor_mul(v2sb[:, kt, :], vsb[:, kt, :],
                                     vsb[:, kt, :])

        # ---- flash main loop ----
        for qt in range(NQ):
            qsl = slice(qt * Q_TILE, (qt + 1) * Q_TILE)
            ckq = ckpool.tile([128, CC, Q_TILE], F16, tag="ckq")
            for c in range(CC):
                nc.sync.dma_start(out=ckq[:, c, :], in_=ckr[c][:, qsl])

            acc1 = [ps_acc.tile([128, 512], F32, tag=f"acc1_{i}",
                                name=f"acc1_{i}") for i in range(NB)]
            acc2 = [ps_acc.tile([128, 512], F32, tag=f"acc2_{i}",
                                name=f"acc2_{i}") for i in range(NB)]
            dps = ps_d.tile([1, Q_TILE], F32, tag="d")

            def acc_ap(accs, c):
                return accs[c // 2][:, (c % 2) * Q_TILE:(c % 2 + 1) * Q_TILE]

            # NOTE: start=True clears has_written bits for the WHOLE psum
            # bank, so each bank (2 c-chunks) forms a single accumulation
            # group: only its first matmul sets start.
            def emit_pv(kt, p):
                nc.tensor.matmul(dps, lhsT=ones_k, rhs=p,
                                 start=(kt == 0), stop=(kt == NK - 1),
                                 skip_group_check=True)
                for acc, lhs in ((acc1, vsb[:, kt, :]), (acc2, v2sb[:, kt, :])):
                    for c in range(CC):
                        csl = slice(c * 128, (c + 1) * 128)
                        nc.tensor.matmul(acc_ap(acc, c),
                                         lhsT=lhs[:, csl],
                                         rhs=p,
                                         start=(kt == 0 and c % 2 == 0),
                                         stop=(kt == NK - 1 and
                                               (c % 2 == 1 or c == CC - 1)),
                                         skip_group_check=True)

            # software pipeline: QK(kt) is emitted before PV(kt-1) so the PE
            # has score matmuls to run while ScalarE computes exp(kt-1).
            pending = []
            for kt in range(NK):
                ksl = slice(kt * 128, (kt + 1) * 128)
                st = ps_st.tile([128, Q_TILE], F32, tag="st")
                for c in range(CC):
                    nc.tensor.matmul(st,
                                     lhsT=g2[:, c, ksl],
                                     rhs=ckq[:, c, :],
                                     start=(c == 0),
                                     stop=(c == CC - 1 and not with_score_bias))
                if with_score_bias:
                    nc.tensor.matmul(st, lhsT=r_sb[:, ksl],
                                     rhs=ones_r[:, :Q_TILE],
                                     start=False, stop=True,
                                     skip_group_check=True)
                p = ppool.tile([128, Q_TILE], F32R, tag="p")
                nc.scalar.activation(out=p, in_=st, func=AF.Exp, bias=shift_sb)
                pending.append((kt, p))
                if len(pending) > 2:
                    emit_pv(*pending.pop(0))
            for item in pending:
                emit_pv(*item)

            # ---- epilogue for this q_tile ----
            rd = epool.tile([1, Q_TILE], F32, tag="rd", bufs=1)
            nc.vector.reciprocal(out=rd, in_=dps)
            rd_dram = dpool.tile([1, Q_TILE], F32, tag="rd_dram")
            nc.sync.dma_start(out=rd_dram, in_=rd)
            rdb = epool.tile([128, Q_TILE], F32, tag="rdb", bufs=1)
            nc.sync.dma_start(out=rdb,
                              in_=rd_dram.to_broadcast([128, Q_TILE]))

            avs, a2s = [], []
            for c in range(CC):
                av = epool.tile([128, Q_TILE], F32, tag=f"av{c}",
                                name=f"av{c}", bufs=1)
                nc.scalar.copy(out=av, in_=acc_ap(acc1, c))
                a2 = epool.tile([128, Q_TILE], F32, tag=f"a2{c}",
                                name=f"a2{c}", bufs=1)
                nc.scalar.copy(out=a2, in_=acc_ap(acc2, c))
                avs.append(av)
                a2s.append(a2)

            for c in range(CC):
                # content ships int8; its per-channel scale is folded into
                # the shipped mu/rstd, so a plain cast suffices.
                ctq = epool.tile([128, Q_TILE], mybir.dt.int8, tag="ctq")
                nc.sync.dma_start(out=ctq, in_=ctr[c][:, qsl])
                ctf = epool.tile([128, Q_TILE], F32, tag="ctf", bufs=1)
                nc.scalar.copy(out=ctf, in_=ctq)
                mean = avs[c]
                nc.vector.tensor_mul(mean, avs[c], rdb)
                e2 = a2s[c]
                nc.vector.tensor_mul(e2, a2s[c], rdb)
                var = epool.tile([128, Q_TILE], F32, tag="var", bufs=1)
                nc.vector.tensor_mul(var, mean, mean)
                nc.vector.scalar_tensor_tensor(
                    out=var, in0=var, scalar=-1.0, in1=e2,
                    op0=ALU.mult, op1=ALU.add)
                nc.vector.tensor_scalar_max(var, var, 1e-38)
                std = var
                nc.scalar.activation(out=std, in_=var, func=AF.Ln)
                nc.scalar.activation(out=std, in_=std, func=AF.Exp, scale=0.5)
                normc = epool.tile([128, Q_TILE], F32, tag="normc", bufs=1)
                nc.vector.tensor_scalar(
                    out=normc, in0=ctf,
                    scalar1=mu[:, c:c + 1], scalar2=rstd[:, c:c + 1],
                    op0=ALU.subtract, op1=ALU.mult)
                o = opool.tile([128, Q_TILE], F32, tag="o")
                nc.vector.tensor_mul(o, std, normc)
                o16 = opool.tile([128, Q_TILE], F16, tag="o16")
                nc.vector.tensor_add(o16, o, mean)
                nc.sync.dma_start(out=outr[c][:, qsl], in_=o16)

    # Force exp/ln/copy onto the shared natural_log_exp_and_others table
    # set: the default per-function choice alternates exp_and_others <->
    # natural_log, costing ~2.7us per ACT_TABLE_LOAD, dozens of times.
    import concourse.bacc as bacc_mod
    _orig_tables = bacc_mod.get_activation_tables
    _keep = "natural_log_exp_and_others"
    _strip = {AF.Exp, AF.Ln, AF.Copy, AF.Identity}

    def _patched_tables(arch):
        t = _orig_tables(arch)
        for name, fns in t.items():
            if name != _keep:
                t[name] = fns - _strip
        return t

    bacc_mod.get_activation_tables = _patched_tables
    try:
        nc.compile()
    finally:
        bacc_mod.get_activation_tables = _orig_tables
    return nc


_PROGRAM_CACHE = {}
_EXEC_CACHE = {}


def _get_program(key):
    if key not in _PROGRAM_CACHE:
        with_r, with_hb = key
        _PROGRAM_CACHE[key] = build_program(
            with_score_bias=with_r, with_v_bias=with_hb)
    return _PROGRAM_CACHE[key]


def prep_inputs(content, style, content_key, style_key, f_w, f_b, g_w, g_b,
                h_w, h_b):
    with_r = bool(np.any(np.asarray(f_b)))
    with_hb = bool(np.any(np.asarray(h_b)))
    key = (with_r, with_hb)

    ct32 = np.asarray(content, np.float32).reshape(B, C, HW)
    ck32 = np.asarray(content_key, np.float32).reshape(B, C, HW)
    sk32 = np.asarray(style_key, np.float32).reshape(B, C, HW)
    sty32 = np.asarray(style, np.float32).reshape(B, C, HW)

    f_w32 = np.asarray(f_w, np.float32)
    g_w32 = np.asarray(g_w, np.float32)
    wT16 = (g_w32.T @ f_w32).astype(np.float16)               # [C, C]
    hwT32 = np.ascontiguousarray(np.asarray(h_w, np.float32).T)
    u = (g_w32.T.astype(np.float64) @ np.asarray(f_b, np.float64)
         if with_r else None)
    hb16 = np.asarray(h_b, np.float16) if with_hb else None
    return key, (key, ct32, ck32, sk32, sty32, wT16, hwT32, u, hb16)


def _quant8(x):
    """Symmetric per-row int8 quantization of [C, N]; returns (q, scale)."""
    s = np.abs(x).max(axis=1) / 127.0
    s = np.maximum(s, 1e-30)
    q = np.clip(np.rint(x * (1.0 / s)[:, None]), -127, 127).astype(np.int8)
    return q, s


def make_blob(b, key, ct32, ck32, sk32, sty32, wT16, hwT32, u, hb16):
    with_r, with_hb = key
    offs, total = _blob_layout(with_r, with_hb)
    blob = np.empty(total, np.uint8)

    def put(name, arr, np_dt):
        o, n, isz = offs[name]
        blob[o:o + n * isz].view(np_dt)[:] = arr.reshape(-1)

    put("ck", ck32[b], np.float16)            # fp32 -> fp16 on assignment
    put("sk", sk32[b], np.float16)
    put("wT", wT16, np.float16)

    # style ships int8; fold its per-channel scale into hwT's rows
    sty_q, sty_s = _quant8(sty32[b])
    put("sty", sty_q, np.int8)
    put("hwT", hwT32 * sty_s[:, None], np.float16)

    # content ships int8; fold its scale into the shipped mu/rstd
    ctb = ct32[b]
    ct_q, ct_s = _quant8(ctb)
    put("ct", ct_q, np.int8)
    mu = ctb.mean(axis=1)                                      # [C]
    var = ctb.var(axis=1, ddof=1) + 1e-5
    rstd = 1.0 / np.sqrt(var)
    mu_f = mu / ct_s                          # device computes (q-mu')*rstd'
    rstd_f = rstd * ct_s
    # stats layout [128, 2*CC]: col c = mu chunk c, col CC+c = rstd chunk
    st = np.empty((128, 2 * CC), np.float16)
    for c in range(CC):
        st[:, c] = mu_f[c * 128:(c + 1) * 128]
        st[:, CC + c] = rstd_f[c * 128:(c + 1) * 128]
    put("stats", st, np.float16)
    if with_r:
        put("rbias", (u @ sk32[b].astype(np.float64)).astype(np.float16),
            np.float16)
    if with_hb:
        put("hb", hb16, np.float16)
    return blob


def make_in_maps(content, style, content_key, style_key, f_w, f_b, g_w, g_b,
                 h_w, h_b):
    key, prep = prep_inputs(content, style, content_key, style_key, f_w,
                            f_b, g_w, g_b, h_w, h_b)
    in_maps = [{"blob": make_blob(b, *prep)} for b in range(B)]
    return in_maps, key


def _get_exec(key):
    """Cached per-device jitted executors (one per core, no retrace).

    Per-core dispatch (instead of one shard_map call) lets the output
    fetch of core i overlap the input upload of core i+1 — the axon pipe
    is full-duplex, so the D2H leg hides entirely under the H2D stream.
    """
    if key in _EXEC_CACHE:
        return _EXEC_CACHE[key]
    import jax
    import jax.numpy as jnp
    from concourse.bass2jax import (
        _bass_exec_p, install_neuronx_cc_hook, partition_id_tensor)

    install_neuronx_cc_hook()
    nc = _get_program(key)
    assert nc.dbg_addr is None
    pname = nc.partition_id_tensor.name if nc.partition_id_tensor else None

    in_names, out_names, out_avals = [], [], []
    for alloc in nc.m.functions[0].allocations:
        if not isinstance(alloc, mybir.MemoryLocationSet):
            continue
        name = alloc.memorylocations[0].name
        if alloc.kind == "ExternalInput":
            if name != pname:
                in_names.append(name)
        elif alloc.kind == "ExternalOutput":
            out_names.append(name)
            out_avals.append(jax.core.ShapedArray(
                tuple(alloc.tensor_shape), mybir.dt.np(alloc.dtype)))
    assert in_names == ["blob"] and out_names == ["out"]
    all_in_names = in_names + out_names
    if pname is not None:
        all_in_names.append(pname)

    def _body(*args):
        operands = list(args)
        if pname is not None:
            operands.append(partition_id_tensor())
        outs = _bass_exec_p.bind(
            *operands,
            out_avals=tuple(out_avals),
            in_names=tuple(all_in_names),
            out_names=tuple(out_names),
            lowering_input_output_aliases=(),
            sim_require_finite=True,
            sim_require_nnan=True,
            nc=nc,
        )
        return tuple(outs)

    devices = jax.devices()[:N_CORES]
    fns = [jax.jit(_body, donate_argnums=(1,), keep_unused=True)
           for _ in devices]
    zfns = [jax.jit(lambda: jnp.zeros((C, HW), jnp.float16), device=d)
            for d in devices]
    _EXEC_CACHE[key] = (fns, zfns, devices)
    return _EXEC_CACHE[key]


def kernel(**inputs):
    import jax
    key, prep = prep_inputs(**inputs)
    fns, zfns, devices = _get_exec(key)
    outs = []
    for b in range(B):
        blob = make_blob(b, *prep)             # host prep overlaps transfers
        xd = jax.device_put(blob, devices[b])
        (o,) = fns[b](xd, zfns[b]())
        o.copy_to_host_async()
        outs.append(o)
    res = np.empty((B, C, HW), np.float32)
    for b in range(B):
        res[b] = np.asarray(outs[b])           # fetch rides under later H2D
    return res.reshape(B, C, 64, 64)


if __name__ == "__main__":
    rng = np.random.default_rng(0)
    inputs = {
        "content": rng.standard_normal((B, C, 64, 64)).astype(np.float32),
        "style": rng.standard_normal((B, C, 64, 64)).astype(np.float32),
        "content_key": rng.standard_normal((B, C, 64, 64)).astype(np.float32),
        "style_key": rng.standard_normal((B, C, 64, 64)).astype(np.float32),
        "f_w": (rng.standard_normal((C, C)) * 0.02).astype(np.float32),
        "f_b": np.zeros(C, np.float32),
        "g_w": (rng.standard_normal((C, C)) * 0.02).astype(np.float32),
        "g_b": np.zeros(C, np.float32),
        "h_w": (rng.standard_normal((C, C)) * 0.02).astype(np.float32),
        "h_b": np.zeros(C, np.float32),
    }
    t0 = time.time()
    out = kernel(**inputs)
    print("kernel done", out.shape, out.dtype, time.time() - t0)


# revision 18
# speedup vs baseline: 1.6603x; 1.1253x over previous
"""AttnAdaIN Trainium2 kernel (v2 — wire-optimized).

Computation (per batch b):
    F = f_w @ CK ; G = g_w @ SK ; Hh = h_w @ STY   (1x1 convs; biases folded
    or cancelled: per-query score offsets cancel in softmax)
    S = softmax_k(F^T G)          [HW, HW]
    mean = S @ Hh^T ; second = S @ (Hh^T)^2
    std = sqrt(relu(second - mean^2))
    out = std * mvn(content) + mean

End-to-end cost through the axon-proxied PJRT tunnel is dominated by
host<->device bytes (~40 MB/s pipe), so v2 minimizes wire traffic:
  * one batch per core on 4 cores — zero data duplication across cores
    (the 8-core query-split shipped sk/sty twice per batch);
  * everything ships as fp16 (PE matmuls run fp16 at the same 1 cycle/row
    as fp32r; the baseline already truncated operands to 11 mantissa bits);
  * per-channel content mean/var computed host-side (ships 2*C floats
    instead of requiring full content on every core);
  * all per-core inputs packed into ONE flat fp16 blob (one transfer, one
    concat) and the output returns as fp16.
On-chip flow matches the proven v1 flash kernel: scores from fp16
G''=W' SK and fp16 CK tiles; exp with fixed -30 shift on ScalarE into
f32r P (full exponent range keeps tiny probabilities); P·V / P·V^2 in
f32r with PSUM-resident accumulators; denominator via ones-vector matmul.
"""

import sys
import time

for _p in ("/opt/trn_rl_repo", "/opt/trn_rl_repo/concourse"):
    if _p not in sys.path:
        sys.path.insert(0, _p)

import contextlib

import numpy as np

import concourse.bacc as bacc
import concourse.mybir as mybir
import concourse.tile as tile

F32 = mybir.dt.float32
F32R = mybir.dt.float32r
F16 = mybir.dt.float16
AF = mybir.ActivationFunctionType
ALU = mybir.AluOpType

C = 512
HW = 4096
B = 4
N_CORES = 4
CC = C // 128
NK = HW // 128
Q_TILE = 256
NQ = HW // Q_TILE
NB = CC // 2  # psum banks per moment accumulator (2 c-chunks per bank)


def _blob_layout(with_score_bias, with_v_bias):
    """Byte-addressed segments of the per-core uint8 input blob.

    fp16 segments first (keeps 2-byte alignment), int8 last. Values are
    (byte_offset, elem_count, itemsize).
    """
    segs = [("ck", C * HW, 2), ("sk", C * HW, 2), ("wT", C * C, 2),
            ("hwT", C * C, 2), ("stats", 128 * 2 * CC, 2)]
    if with_score_bias:
        segs.append(("rbias", HW, 2))
    if with_v_bias:
        segs.append(("hb", C, 2))
    segs += [("sty", C * HW, 1), ("ct", C * HW, 1)]
    offs, off = {}, 0
    for name, n, isz in segs:
        offs[name] = (off, n, isz)
        off += n * isz
    return offs, off


def build_program(with_score_bias=False, with_v_bias=False):
    offs, total = _blob_layout(with_score_bias, with_v_bias)

    nc = bacc.Bacc("TRN2", target_bir_lowering=False, debug=False,
                   num_devices=N_CORES)

    I8 = mybir.dt.int8
    U8 = mybir.dt.uint8
    blob = nc.dram_tensor("blob", [total], U8, kind="ExternalInput")
    # int8 output + per-channel scales: halves the D2H bytes (the host pipe
    # is CPU-bound, so D2H bytes cost the same budget as H2D bytes).
    out = nc.dram_tensor("out", [C, HW], I8, kind="ExternalOutput")
    oscale = nc.dram_tensor("oscale", [128, CC], F32, kind="ExternalOutput")
    scratch = nc.dram_tensor("scratch", [C, HW], F16, kind="Internal")

    def seg(name, dt):
        o, n, isz = offs[name]
        return blob[o:o + n * isz].bitcast(dt)

    def seg3(name, dt=F16):
        v = seg(name, dt)
        inner = v.shape[0] // (CC * 128)
        return v.rearrange("(c p q) -> c p q", c=CC, p=128, q=inner)

    ckr = seg3("ck")            # [CC, 128, HW] fp16
    skr = seg3("sk")
    styr = seg3("sty", I8)      # [CC, 128, HW] int8
    ctr = seg3("ct", I8)
    wTr = seg3("wT")            # [CC, 128, C] fp16
    hwTr = seg3("hwT")
    statr = seg("stats", F16).rearrange("(p s) -> p s", p=128, s=2 * CC)
    if with_score_bias:
        rbias = seg("rbias", F16).rearrange("(one k) -> one k", one=1, k=HW)
    if with_v_bias:
        hb = seg("hb", F16).rearrange("(one c) -> one c", one=1, c=C)
    outr = out.rearrange("(c p) q -> c p q", p=128)
    scrr = scratch.rearrange("(c p) q -> c p q", p=128)

    with tile.TileContext(nc) as tc, contextlib.ExitStack() as ctx:
        persist = ctx.enter_context(tc.tile_pool(name="persist", bufs=1))
        ckpool = ctx.enter_context(tc.tile_pool(name="ckpool", bufs=2))
        ppool = ctx.enter_context(tc.tile_pool(name="ppool", bufs=4))
        epool = ctx.enter_context(tc.tile_pool(name="epool", bufs=2))
        opool = ctx.enter_context(tc.tile_pool(name="opool", bufs=2))
        ps_st = ctx.enter_context(
            tc.tile_pool(name="ps_st", bufs=3, space="PSUM"))
        ps_acc = ctx.enter_context(
            tc.tile_pool(name="ps_acc", bufs=1, space="PSUM"))
        ps_d = ctx.enter_context(
            tc.tile_pool(name="ps_d", bufs=1, space="PSUM"))
        dpool = ctx.enter_context(
            tc.tile_pool(name="dpool", bufs=2, space="DRAM"))

        # ---- constants (memset, nothing shipped) ----
        # memset only supports 32-bit value types; memset F32 then use a
        # bitcast view for the F32R/F16 matmul operands.
        ones_k_f = persist.tile([128, 1], F32, tag="ones_k")
        nc.vector.memset(ones_k_f, 1.0)
        ones_k = ones_k_f.bitcast(F32R)
        shift_sb = persist.tile([128, 1], F32, tag="shift")
        nc.vector.memset(shift_sb, -30.0)
        if with_score_bias or with_v_bias:
            ones_r_f = persist.tile([1, 64], F32, tag="ones_r")
            nc.vector.memset(ones_r_f, float(
                np.frombuffer(np.array([1.0, 1.0], np.float16).tobytes(),
                              np.float32)[0]))
            ones_r = ones_r_f.bitcast(F16)           # [1, 128] fp16 ones

        g2 = persist.tile([128, CC, HW], F16, tag="g2")
        vsb = persist.tile([128, NK, C], F32R, tag="v")
        v2sb = persist.tile([128, NK, C], F32R, tag="v2")
        mu = persist.tile([128, CC], F32, tag="mu")
        rstd = persist.tile([128, CC], F32, tag="rstd")
        omax = persist.tile([128, CC], F32, tag="omax")
        nc.vector.memset(omax, 1e-30)
        if with_score_bias:
            r_sb = persist.tile([1, HW], F16, tag="rbias")
            nc.sync.dma_start(out=r_sb, in_=rbias[:])
        if with_v_bias:
            hb_sb = persist.tile([1, C], F16, tag="hb")
            nc.sync.dma_start(out=hb_sb, in_=hb[:])

        # ---- phase 0: weights, stats, G'' and V/V^2 precompute ----
        with tc.tile_pool(name="ph0", bufs=1) as ph0, \
             tc.tile_pool(name="ph0s", bufs=2) as ph0s:
            stat_sb = ph0.tile([128, 2 * CC], F16, tag="stats")
            nc.sync.dma_start(out=stat_sb, in_=statr[:])
            nc.scalar.copy(out=mu, in_=stat_sb[:, 0:CC])
            nc.scalar.copy(out=rstd, in_=stat_sb[:, CC:2 * CC])

            wT_sb = ph0.tile([128, CC, C], F16, tag="wT")
            hwT_sb = ph0.tile([128, CC, C], F16, tag="hwT")
            for c in range(CC):
                nc.sync.dma_start(out=wT_sb[:, c, :], in_=wTr[c])
                nc.sync.dma_start(out=hwT_sb[:, c, :], in_=hwTr[c])

            # G'' = W'^T SK  (score stationary operand), layout [c, k], fp16
            for ks in range(HW // 256):
                sl = slice(ks * 256, (ks + 1) * 256)
                sks = ph0s.tile([128, CC, 256], F16, tag="sk_stream")
                for b in range(CC):
                    nc.sync.dma_start(out=sks[:, b, :], in_=skr[b][:, sl])
                for a in range(CC):
                    gps = ps_st.tile([128, 256], F32, tag="st", name="gps")
                    for b in range(CC):
                        nc.tensor.matmul(
                            gps,
                            lhsT=wT_sb[:, b, a * 128:(a + 1) * 128],
                            rhs=sks[:, b, :],
                            start=(b == 0), stop=(b == CC - 1))
                    nc.scalar.copy(out=g2[:, a, sl], in_=gps)

            # V = STY^T hwT  ([k, c] in 128-row blocks) and V^2, f32r.
            # STY ships int8 (per-channel scales folded into hwT host-side);
            # cast int8 -> fp16 on ScalarE before the matmul.
            for kt in range(NK):
                sl = slice(kt * 128, (kt + 1) * 128)
                sts8 = ph0s.tile([128, CC, 128], mybir.dt.int8,
                                 tag="sty_stream8")
                for b in range(CC):
                    nc.sync.dma_start(out=sts8[:, b, :], in_=styr[b][:, sl])
                sts = ph0s.tile([128, CC, 128], F16, tag="sty_stream")
                nc.scalar.copy(out=sts[:, :, :], in_=sts8[:, :, :])
                vps = ps_st.tile([128, 512], F32, tag="st")
                for b in range(CC):
                    nc.tensor.matmul(vps[:, :C],
                                     lhsT=sts[:, b, :],
                                     rhs=hwT_sb[:, b, :],
                                     start=(b == 0), stop=(b == CC - 1))
                if with_v_bias:
                    nc.tensor.matmul(vps[:, :C],
                                     lhsT=ones_r,
                                     rhs=hb_sb,
                                     start=False, stop=True,
                                     skip_group_check=True)
                nc.scalar.copy(out=vsb[:, kt, :], in_=vps[:, :C])
                nc.vector.tensor_mul(v2sb[:, kt, :], vsb[:, kt, :],
                                     vsb[:, kt, :])

        # ---- flash main loop ----
        for qt in range(NQ):
            qsl = slice(qt * Q_TILE, (qt + 1) * Q_TILE)
            ckq = ckpool.tile([128, CC, Q_TILE], F16, tag="ckq")
            for c in range(CC):
                nc.sync.dma_start(out=ckq[:, c, :], in_=ckr[c][:, qsl])

            acc1 = [ps_acc.tile([128, 512], F32, tag=f"acc1_{i}",
                                name=f"acc1_{i}") for i in range(NB)]
            acc2 = [ps_acc.tile([128, 512], F32, tag=f"acc2_{i}",
                                name=f"acc2_{i}") for i in range(NB)]
            dps = ps_d.tile([1, Q_TILE], F32, tag="d")

            def acc_ap(accs, c):
                return accs[c // 2][:, (c % 2) * Q_TILE:(c % 2 + 1) * Q_TILE]

            # NOTE: start=True clears has_written bits for the WHOLE psum
            # bank, so each bank (2 c-chunks) forms a single accumulation
            # group: only its first matmul sets start.
            def emit_pv(kt, p):
                nc.tensor.matmul(dps, lhsT=ones_k, rhs=p,
                                 start=(kt == 0), stop=(kt == NK - 1),
                                 skip_group_check=True)
                for acc, lhs in ((acc1, vsb[:, kt, :]), (acc2, v2sb[:, kt, :])):
                    for c in range(CC):
                        csl = slice(c * 128, (c + 1) * 128)
                        nc.tensor.matmul(acc_ap(acc, c),
                                         lhsT=lhs[:, csl],
                                         rhs=p,
                                         start=(kt == 0 and c % 2 == 0),
                                         stop=(kt == NK - 1 and
                                               (c % 2 == 1 or c == CC - 1)),
                                         skip_group_check=True)

            # software pipeline: QK(kt) is emitted before PV(kt-1) so the PE
            # has score matmuls to run while ScalarE computes exp(kt-1).
            pending = []
            for kt in range(NK):
                ksl = slice(kt * 128, (kt + 1) * 128)
                st = ps_st.tile([128, Q_TILE], F32, tag="st")
                for c in range(CC):
                    nc.tensor.matmul(st,
                                     lhsT=g2[:, c, ksl],
                                     rhs=ckq[:, c, :],
                                     start=(c == 0),
                                     stop=(c == CC - 1 and not with_score_bias))
                if with_score_bias:
                    nc.tensor.matmul(st, lhsT=r_sb[:, ksl],
                                     rhs=ones_r[:, :Q_TILE],
                                     start=False, stop=True,
                                     skip_group_check=True)
                p = ppool.tile([128, Q_TILE], F32R, tag="p")
                nc.scalar.activation(out=p, in_=st, func=AF.Exp, bias=shift_sb)
                pending.append((kt, p))
                if len(pending) > 2:
                    emit_pv(*pending.pop(0))
            for item in pending:
                emit_pv(*item)

            # ---- epilogue for this q_tile ----
            rd = epool.tile([1, Q_TILE], F32, tag="rd", bufs=1)
            nc.vector.reciprocal(out=rd, in_=dps)
            rd_dram = dpool.tile([1, Q_TILE], F32, tag="rd_dram")
            nc.sync.dma_start(out=rd_dram, in_=rd)
            rdb = epool.tile([128, Q_TILE], F32, tag="rdb", bufs=1)
            nc.sync.dma_start(out=rdb,
                              in_=rd_dram.to_broadcast([128, Q_TILE]))

            avs, a2s = [], []
            for c in range(CC):
                av = epool.tile([128, Q_TILE], F32, tag=f"av{c}",
                                name=f"av{c}", bufs=1)
                nc.scalar.copy(out=av, in_=acc_ap(acc1, c))
                a2 = epool.tile([128, Q_TILE], F32, tag=f"a2{c}",
                                name=f"a2{c}", bufs=1)
                nc.scalar.copy(out=a2, in_=acc_ap(acc2, c))
                avs.append(av)
                a2s.append(a2)

            for c in range(CC):
                # content ships int8; its per-channel scale is folded into
                # the shipped mu/rstd, so a plain cast suffices.
                ctq = epool.tile([128, Q_TILE], mybir.dt.int8, tag="ctq")
                nc.sync.dma_start(out=ctq, in_=ctr[c][:, qsl])
                ctf = epool.tile([128, Q_TILE], F32, tag="ctf", bufs=1)
                nc.scalar.copy(out=ctf, in_=ctq)
                mean = avs[c]
                nc.vector.tensor_mul(mean, avs[c], rdb)
                e2 = a2s[c]
                nc.vector.tensor_mul(e2, a2s[c], rdb)
                var = epool.tile([128, Q_TILE], F32, tag="var", bufs=1)
                nc.vector.tensor_mul(var, mean, mean)
                nc.vector.scalar_tensor_tensor(
                    out=var, in0=var, scalar=-1.0, in1=e2,
                    op0=ALU.mult, op1=ALU.add)
                nc.vector.tensor_scalar_max(var, var, 1e-38)
                std = var
                nc.scalar.activation(out=std, in_=var, func=AF.Ln)
                nc.scalar.activation(out=std, in_=std, func=AF.Exp, scale=0.5)
                normc = epool.tile([128, Q_TILE], F32, tag="normc", bufs=1)
                nc.vector.tensor_scalar(
                    out=normc, in0=ctf,
                    scalar1=mu[:, c:c + 1], scalar2=rstd[:, c:c + 1],
                    op0=ALU.subtract, op1=ALU.mult)
                o = opool.tile([128, Q_TILE], F32, tag="o")
                nc.vector.tensor_mul(o, std, normc)
                o16 = opool.tile([128, Q_TILE], F16, tag="o16")
                nc.vector.tensor_add(o16, o, mean)
                om = epool.tile([128, 1], F32, tag="om", bufs=1)
                nc.vector.tensor_reduce(out=om, in_=o16,
                                        axis=mybir.AxisListType.X,
                                        op=ALU.max, apply_absolute_value=True)
                nc.vector.tensor_max(omax[:, c:c + 1], omax[:, c:c + 1], om)
                nc.sync.dma_start(out=scrr[c][:, qsl], in_=o16)

        # ---- pass 2: quantize the fp16 scratch to int8 with per-channel
        # scales (scale = max|out| / 127, computed above) ----
        qsc = persist.tile([128, CC], F32, tag="qsc")
        nc.vector.reciprocal(out=qsc, in_=omax)
        nc.vector.tensor_scalar_mul(qsc, qsc, 127.0)
        osc = persist.tile([128, CC], F32, tag="osc")
        nc.vector.tensor_scalar_mul(osc, omax, 1.0 / 127.0)
        nc.sync.dma_start(out=oscale[:], in_=osc)
        QW = 512
        for c in range(CC):
            for j in range(HW // QW):
                jsl = slice(j * QW, (j + 1) * QW)
                sof = opool.tile([128, QW], F16, tag="sof")
                nc.sync.dma_start(out=sof, in_=scrr[c][:, jsl])
                q8 = opool.tile([128, QW], mybir.dt.int8, tag="q8")
                nc.vector.tensor_scalar_mul(q8, sof, qsc[:, c:c + 1])
                nc.sync.dma_start(out=outr[c][:, jsl], in_=q8)

    # Force exp/ln/copy onto the shared natural_log_exp_and_others table
    # set: the default per-function choice alternates exp_and_others <->
    # natural_log, costing ~2.7us per ACT_TABLE_LOAD, dozens of times.
    import concourse.bacc as bacc_mod
    _orig_tables = bacc_mod.get_activation_tables
    _keep = "natural_log_exp_and_others"
    _strip = {AF.Exp, AF.Ln, AF.Copy, AF.Identity}

    def _patched_tables(arch):
        t = _orig_tables(arch)
        for name, fns in t.items():
            if name != _keep:
                t[name] = fns - _strip
        return t

    bacc_mod.get_activation_tables = _patched_tables
    try:
        nc.compile()
    finally:
        bacc_mod.get_activation_tables = _orig_tables
    return nc


_PROGRAM_CACHE = {}
_EXEC_CACHE = {}


def _get_program(key):
    if key not in _PROGRAM_CACHE:
        with_r, with_hb = key
        _PROGRAM_CACHE[key] = build_program(
            with_score_bias=with_r, with_v_bias=with_hb)
    return _PROGRAM_CACHE[key]


def prep_inputs(content, style, content_key, style_key, f_w, f_b, g_w, g_b,
                h_w, h_b):
    with_r = bool(np.any(np.asarray(f_b)))
    with_hb = bool(np.any(np.asarray(h_b)))
    key = (with_r, with_hb)

    ct32 = np.asarray(content, np.float32).reshape(B, C, HW)
    ck32 = np.asarray(content_key, np.float32).reshape(B, C, HW)
    sk32 = np.asarray(style_key, np.float32).reshape(B, C, HW)
    sty32 = np.asarray(style, np.float32).reshape(B, C, HW)

    f_w32 = np.asarray(f_w, np.float32)
    g_w32 = np.asarray(g_w, np.float32)
    wT16 = (g_w32.T @ f_w32).astype(np.float16)               # [C, C]
    hwT32 = np.ascontiguousarray(np.asarray(h_w, np.float32).T)
    u = (g_w32.T.astype(np.float64) @ np.asarray(f_b, np.float64)
         if with_r else None)
    hb16 = np.asarray(h_b, np.float16) if with_hb else None
    return key, (key, ct32, ck32, sk32, sty32, wT16, hwT32, u, hb16)


def _quant8(x):
    """Symmetric per-row int8 quantization of [C, N]; returns (q, scale)."""
    s = np.abs(x).max(axis=1) / 127.0
    s = np.maximum(s, 1e-30)
    q = np.clip(np.rint(x * (1.0 / s)[:, None]), -127, 127).astype(np.int8)
    return q, s


def make_blob(b, key, ct32, ck32, sk32, sty32, wT16, hwT32, u, hb16):
    with_r, with_hb = key
    offs, total = _blob_layout(with_r, with_hb)
    blob = np.empty(total, np.uint8)

    def put(name, arr, np_dt):
        o, n, isz = offs[name]
        blob[o:o + n * isz].view(np_dt)[:] = arr.reshape(-1)

    put("ck", ck32[b], np.float16)            # fp32 -> fp16 on assignment
    put("sk", sk32[b], np.float16)
    put("wT", wT16, np.float16)

    # style ships int8; fold its per-channel scale into hwT's rows
    sty_q, sty_s = _quant8(sty32[b])
    put("sty", sty_q, np.int8)
    put("hwT", hwT32 * sty_s[:, None], np.float16)

    # content ships int8; fold its scale into the shipped mu/rstd
    ctb = ct32[b]
    ct_q, ct_s = _quant8(ctb)
    put("ct", ct_q, np.int8)
    mu = ctb.mean(axis=1)                                      # [C]
    var = ctb.var(axis=1, ddof=1) + 1e-5
    rstd = 1.0 / np.sqrt(var)
    mu_f = mu / ct_s                          # device computes (q-mu')*rstd'
    rstd_f = rstd * ct_s
    # stats layout [128, 2*CC]: col c = mu chunk c, col CC+c = rstd chunk
    st = np.empty((128, 2 * CC), np.float16)
    for c in range(CC):
        st[:, c] = mu_f[c * 128:(c + 1) * 128]
        st[:, CC + c] = rstd_f[c * 128:(c + 1) * 128]
    put("stats", st, np.float16)
    if with_r:
        put("rbias", (u @ sk32[b].astype(np.float64)).astype(np.float16),
            np.float16)
    if with_hb:
        put("hb", hb16, np.float16)
    return blob


def make_in_maps(content, style, content_key, style_key, f_w, f_b, g_w, g_b,
                 h_w, h_b):
    key, prep = prep_inputs(content, style, content_key, style_key, f_w,
                            f_b, g_w, g_b, h_w, h_b)
    in_maps = [{"blob": make_blob(b, *prep)} for b in range(B)]
    return in_maps, key


def _get_exec(key):
    """Cached per-device jitted executors (one per core, no retrace).

    Per-core dispatch (instead of one shard_map call) lets the output
    fetch of core i overlap the input upload of core i+1 — the axon pipe
    is full-duplex, so the D2H leg hides entirely under the H2D stream.
    """
    if key in _EXEC_CACHE:
        return _EXEC_CACHE[key]
    import jax
    import jax.numpy as jnp
    from concourse.bass2jax import (
        _bass_exec_p, install_neuronx_cc_hook, partition_id_tensor)

    install_neuronx_cc_hook()
    nc = _get_program(key)
    assert nc.dbg_addr is None
    pname = nc.partition_id_tensor.name if nc.partition_id_tensor else None

    in_names, out_names, out_avals = [], [], []
    for alloc in nc.m.functions[0].allocations:
        if not isinstance(alloc, mybir.MemoryLocationSet):
            continue
        name = alloc.memorylocations[0].name
        if alloc.kind == "ExternalInput":
            if name != pname:
                in_names.append(name)
        elif alloc.kind == "ExternalOutput":
            out_names.append(name)
            out_avals.append(jax.core.ShapedArray(
                tuple(alloc.tensor_shape), mybir.dt.np(alloc.dtype)))
    assert in_names == ["blob"] and out_names == ["out", "oscale"]
    # Outputs are fully written by the kernel, so no pre-zeroed donated
    # output operands are needed (they'd cost an extra dispatch each call).
    all_in_names = list(in_names)
    if pname is not None:
        all_in_names.append(pname)

    def _body(*args):
        operands = list(args)
        if pname is not None:
            operands.append(partition_id_tensor())
        outs = _bass_exec_p.bind(
            *operands,
            out_avals=tuple(out_avals),
            in_names=tuple(all_in_names),
            out_names=tuple(out_names),
            lowering_input_output_aliases=(),
            sim_require_finite=True,
            sim_require_nnan=True,
            nc=nc,
        )
        return tuple(outs)

    devices = jax.devices()[:N_CORES]
    fns = [jax.jit(_body, keep_unused=True) for _ in devices]
    _EXEC_CACHE[key] = (fns, devices)
    return _EXEC_CACHE[key]


def kernel(**inputs):
    import jax
    key, prep = prep_inputs(**inputs)
    fns, devices = _get_exec(key)
    outs = []
    for b in range(B):
        blob = make_blob(b, *prep)             # host prep overlaps transfers
        xd = jax.device_put(blob, devices[b])
        o, osc = fns[b](xd)
        o.copy_to_host_async()
        osc.copy_to_host_async()
        outs.append((o, osc))
    res = np.empty((B, C, HW), np.float32)
    for b in range(B):
        o, osc = outs[b]
        q = np.asarray(o)                      # [C, HW] int8
        s = np.asarray(osc).T.reshape(C)       # [128, CC] -> per-channel
        np.multiply(q, s[:, None], out=res[b], casting="unsafe")
    return res.reshape(B, C, 64, 64)


if __name__ == "__main__":
    rng = np.random.default_rng(0)
    inputs = {
        "content": rng.standard_normal((B, C, 64, 64)).astype(np.float32),
        "style": rng.standard_normal((B, C, 64, 64)).astype(np.float32),
        "content_key": rng.standard_normal((B, C, 64, 64)).astype(np.float32),
        "style_key": rng.standard_normal((B, C, 64, 64)).astype(np.float32),
        "f_w": (rng.standard_normal((C, C)) * 0.02).astype(np.float32),
        "f_b": np.zeros(C, np.float32),
        "g_w": (rng.standard_normal((C, C)) * 0.02).astype(np.float32),
        "g_b": np.zeros(C, np.float32),
        "h_w": (rng.standard_normal((C, C)) * 0.02).astype(np.float32),
        "h_b": np.zeros(C, np.float32),
    }
    t0 = time.time()
    out = kernel(**inputs)
    print("kernel done", out.shape, out.dtype, time.time() - t0)


# revision 25
# speedup vs baseline: 1.7254x; 1.0392x over previous
"""AttnAdaIN Trainium2 kernel (v2 — wire-optimized).

Computation (per batch b):
    F = f_w @ CK ; G = g_w @ SK ; Hh = h_w @ STY   (1x1 convs; biases folded
    or cancelled: per-query score offsets cancel in softmax)
    S = softmax_k(F^T G)          [HW, HW]
    mean = S @ Hh^T ; second = S @ (Hh^T)^2
    std = sqrt(relu(second - mean^2))
    out = std * mvn(content) + mean

End-to-end cost through the axon-proxied PJRT tunnel is dominated by
host<->device bytes (~40 MB/s pipe), so v2 minimizes wire traffic:
  * one batch per core on 4 cores — zero data duplication across cores
    (the 8-core query-split shipped sk/sty twice per batch);
  * everything ships as fp16 (PE matmuls run fp16 at the same 1 cycle/row
    as fp32r; the baseline already truncated operands to 11 mantissa bits);
  * per-channel content mean/var computed host-side (ships 2*C floats
    instead of requiring full content on every core);
  * all per-core inputs packed into ONE flat fp16 blob (one transfer, one
    concat) and the output returns as fp16.
On-chip flow matches the proven v1 flash kernel: scores from fp16
G''=W' SK and fp16 CK tiles; exp with fixed -30 shift on ScalarE into
f32r P (full exponent range keeps tiny probabilities); P·V / P·V^2 in
f32r with PSUM-resident accumulators; denominator via ones-vector matmul.
"""

import sys
import time

for _p in ("/opt/trn_rl_repo", "/opt/trn_rl_repo/concourse"):
    if _p not in sys.path:
        sys.path.insert(0, _p)

import contextlib

import numpy as np

import concourse.bacc as bacc
import concourse.mybir as mybir
import concourse.tile as tile

F32 = mybir.dt.float32
F32R = mybir.dt.float32r
F16 = mybir.dt.float16
AF = mybir.ActivationFunctionType
ALU = mybir.AluOpType

C = 512
HW = 4096
B = 4
N_CORES = 4
CC = C // 128
NK = HW // 128
Q_TILE = 256
NQ = HW // Q_TILE
NB = CC // 2  # psum banks per moment accumulator (2 c-chunks per bank)


def _blob_layout(with_score_bias, with_v_bias):
    """Byte-addressed segments of the per-core uint8 input blob.

    fp16 segments first (keeps 2-byte alignment), int8 last. Values are
    (byte_offset, elem_count, itemsize).
    """
    segs = [("ck", C * HW, 2), ("sk", C * HW, 2), ("wT", C * C, 2),
            ("hwT", C * C, 2), ("stats", 128 * 2 * CC, 2)]
    if with_score_bias:
        segs.append(("rbias", HW, 2))
    if with_v_bias:
        segs.append(("hb", C, 2))
    segs += [("sty", C * HW, 1), ("ct", C * HW, 1)]
    offs, off = {}, 0
    for name, n, isz in segs:
        offs[name] = (off, n, isz)
        off += n * isz
    return offs, off


def build_program(with_score_bias=False, with_v_bias=False):
    offs, total = _blob_layout(with_score_bias, with_v_bias)

    nc = bacc.Bacc("TRN2", target_bir_lowering=False, debug=False,
                   num_devices=N_CORES)

    I8 = mybir.dt.int8
    U8 = mybir.dt.uint8
    blob = nc.dram_tensor("blob", [total], U8, kind="ExternalInput")
    # int8 output + per-channel scales: halves the D2H bytes (the host pipe
    # is CPU-bound, so D2H bytes cost the same budget as H2D bytes).
    out = nc.dram_tensor("out", [C, HW], I8, kind="ExternalOutput")
    oscale = nc.dram_tensor("oscale", [128, CC], F32, kind="ExternalOutput")
    scratch = nc.dram_tensor("scratch", [C, HW], F16, kind="Internal")

    def seg(name, dt):
        o, n, isz = offs[name]
        return blob[o:o + n * isz].bitcast(dt)

    def seg3(name, dt=F16):
        v = seg(name, dt)
        inner = v.shape[0] // (CC * 128)
        return v.rearrange("(c p q) -> c p q", c=CC, p=128, q=inner)

    ckr = seg3("ck")            # [CC, 128, HW] fp16
    skr = seg3("sk")
    styr = seg3("sty", U8)      # [CC, 128, HW] uint8 (offset 127)
    ctr = seg3("ct", U8)
    wTr = seg3("wT")            # [CC, 128, C] fp16
    hwTr = seg3("hwT")
    statr = seg("stats", F16).rearrange("(p s) -> p s", p=128, s=2 * CC)
    if with_score_bias:
        rbias = seg("rbias", F16).rearrange("(one k) -> one k", one=1, k=HW)
    if with_v_bias:
        hb = seg("hb", F16).rearrange("(one c) -> one c", one=1, c=C)
    outr = out.rearrange("(c p) q -> c p q", p=128)
    scrr = scratch.rearrange("(c p) q -> c p q", p=128)

    with tile.TileContext(nc) as tc, contextlib.ExitStack() as ctx:
        persist = ctx.enter_context(tc.tile_pool(name="persist", bufs=1))
        ckpool = ctx.enter_context(tc.tile_pool(name="ckpool", bufs=2))
        ppool = ctx.enter_context(tc.tile_pool(name="ppool", bufs=4))
        epool = ctx.enter_context(tc.tile_pool(name="epool", bufs=2))
        opool = ctx.enter_context(tc.tile_pool(name="opool", bufs=2))
        ps_st = ctx.enter_context(
            tc.tile_pool(name="ps_st", bufs=3, space="PSUM"))
        ps_acc = ctx.enter_context(
            tc.tile_pool(name="ps_acc", bufs=1, space="PSUM"))
        ps_d = ctx.enter_context(
            tc.tile_pool(name="ps_d", bufs=1, space="PSUM"))
        dpool = ctx.enter_context(
            tc.tile_pool(name="dpool", bufs=2, space="DRAM"))

        # ---- constants (memset, nothing shipped) ----
        # memset only supports 32-bit value types; memset F32 then use a
        # bitcast view for the F32R/F16 matmul operands.
        ones_k_f = persist.tile([128, 1], F32, tag="ones_k")
        nc.vector.memset(ones_k_f, 1.0)
        ones_k = ones_k_f.bitcast(F32R)
        shift_sb = persist.tile([128, 1], F32, tag="shift")
        nc.vector.memset(shift_sb, -30.0)
        if with_score_bias or with_v_bias:
            ones_r_f = persist.tile([1, 64], F32, tag="ones_r")
            nc.vector.memset(ones_r_f, float(
                np.frombuffer(np.array([1.0, 1.0], np.float16).tobytes(),
                              np.float32)[0]))
            ones_r = ones_r_f.bitcast(F16)           # [1, 128] fp16 ones

        g2 = persist.tile([128, CC, HW], F16, tag="g2")
        vsb = persist.tile([128, NK, C], F32R, tag="v")
        v2sb = persist.tile([128, NK, C], F32R, tag="v2")
        mu = persist.tile([128, CC], F32, tag="mu")
        rstd = persist.tile([128, CC], F32, tag="rstd")
        omax = persist.tile([128, CC], F32, tag="omax")
        nc.vector.memset(omax, 1e-30)
        if with_score_bias:
            r_sb = persist.tile([1, HW], F16, tag="rbias")
            nc.sync.dma_start(out=r_sb, in_=rbias[:])
        if with_v_bias:
            hb_sb = persist.tile([1, C], F16, tag="hb")
            nc.sync.dma_start(out=hb_sb, in_=hb[:])

        # ---- phase 0: weights, stats, G'' and V/V^2 precompute ----
        with tc.tile_pool(name="ph0", bufs=1) as ph0, \
             tc.tile_pool(name="ph0s", bufs=2) as ph0s:
            stat_sb = ph0.tile([128, 2 * CC], F16, tag="stats")
            nc.sync.dma_start(out=stat_sb, in_=statr[:])
            nc.scalar.copy(out=mu, in_=stat_sb[:, 0:CC])
            nc.scalar.copy(out=rstd, in_=stat_sb[:, CC:2 * CC])

            wT_sb = ph0.tile([128, CC, C], F16, tag="wT")
            hwT_sb = ph0.tile([128, CC, C], F16, tag="hwT")
            for c in range(CC):
                nc.sync.dma_start(out=wT_sb[:, c, :], in_=wTr[c])
                nc.sync.dma_start(out=hwT_sb[:, c, :], in_=hwTr[c])

            # G'' = W'^T SK  (score stationary operand), layout [c, k], fp16
            for ks in range(HW // 256):
                sl = slice(ks * 256, (ks + 1) * 256)
                sks = ph0s.tile([128, CC, 256], F16, tag="sk_stream")
                for b in range(CC):
                    nc.sync.dma_start(out=sks[:, b, :], in_=skr[b][:, sl])
                for a in range(CC):
                    gps = ps_st.tile([128, 256], F32, tag="st", name="gps")
                    for b in range(CC):
                        nc.tensor.matmul(
                            gps,
                            lhsT=wT_sb[:, b, a * 128:(a + 1) * 128],
                            rhs=sks[:, b, :],
                            start=(b == 0), stop=(b == CC - 1))
                    nc.scalar.copy(out=g2[:, a, sl], in_=gps)

            # V = STY^T hwT  ([k, c] in 128-row blocks) and V^2, f32r.
            # STY ships uint8 (offset 127, per-channel scales folded into
            # hwT host-side); the ScalarE cast applies the -127 recenter.
            for kt in range(NK):
                sl = slice(kt * 128, (kt + 1) * 128)
                sts8 = ph0s.tile([128, CC, 128], mybir.dt.uint8,
                                 tag="sty_stream8")
                for b in range(CC):
                    nc.sync.dma_start(out=sts8[:, b, :], in_=styr[b][:, sl])
                sts = ph0s.tile([128, CC, 128], F16, tag="sty_stream")
                nc.scalar.activation(out=sts[:, :, :], in_=sts8[:, :, :],
                                     func=AF.Copy, bias=-127.0)
                vps = ps_st.tile([128, 512], F32, tag="st")
                for b in range(CC):
                    nc.tensor.matmul(vps[:, :C],
                                     lhsT=sts[:, b, :],
                                     rhs=hwT_sb[:, b, :],
                                     start=(b == 0), stop=(b == CC - 1))
                if with_v_bias:
                    nc.tensor.matmul(vps[:, :C],
                                     lhsT=ones_r,
                                     rhs=hb_sb,
                                     start=False, stop=True,
                                     skip_group_check=True)
                nc.scalar.copy(out=vsb[:, kt, :], in_=vps[:, :C])
                nc.vector.tensor_mul(v2sb[:, kt, :], vsb[:, kt, :],
                                     vsb[:, kt, :])

        # ---- flash main loop ----
        for qt in range(NQ):
            qsl = slice(qt * Q_TILE, (qt + 1) * Q_TILE)
            ckq = ckpool.tile([128, CC, Q_TILE], F16, tag="ckq")
            for c in range(CC):
                nc.sync.dma_start(out=ckq[:, c, :], in_=ckr[c][:, qsl])

            acc1 = [ps_acc.tile([128, 512], F32, tag=f"acc1_{i}",
                                name=f"acc1_{i}") for i in range(NB)]
            acc2 = [ps_acc.tile([128, 512], F32, tag=f"acc2_{i}",
                                name=f"acc2_{i}") for i in range(NB)]
            dps = ps_d.tile([1, Q_TILE], F32, tag="d")

            def acc_ap(accs, c):
                return accs[c // 2][:, (c % 2) * Q_TILE:(c % 2 + 1) * Q_TILE]

            # NOTE: start=True clears has_written bits for the WHOLE psum
            # bank, so each bank (2 c-chunks) forms a single accumulation
            # group: only its first matmul sets start.
            def emit_pv(kt, p):
                nc.tensor.matmul(dps, lhsT=ones_k, rhs=p,
                                 start=(kt == 0), stop=(kt == NK - 1),
                                 skip_group_check=True)
                for acc, lhs in ((acc1, vsb[:, kt, :]), (acc2, v2sb[:, kt, :])):
                    for c in range(CC):
                        csl = slice(c * 128, (c + 1) * 128)
                        nc.tensor.matmul(acc_ap(acc, c),
                                         lhsT=lhs[:, csl],
                                         rhs=p,
                                         start=(kt == 0 and c % 2 == 0),
                                         stop=(kt == NK - 1 and
                                               (c % 2 == 1 or c == CC - 1)),
                                         skip_group_check=True)

            # software pipeline: QK(kt) is emitted before PV(kt-1) so the PE
            # has score matmuls to run while ScalarE computes exp(kt-1).
            pending = []
            for kt in range(NK):
                ksl = slice(kt * 128, (kt + 1) * 128)
                st = ps_st.tile([128, Q_TILE], F32, tag="st")
                for c in range(CC):
                    nc.tensor.matmul(st,
                                     lhsT=g2[:, c, ksl],
                                     rhs=ckq[:, c, :],
                                     start=(c == 0),
                                     stop=(c == CC - 1 and not with_score_bias))
                if with_score_bias:
                    nc.tensor.matmul(st, lhsT=r_sb[:, ksl],
                                     rhs=ones_r[:, :Q_TILE],
                                     start=False, stop=True,
                                     skip_group_check=True)
                p = ppool.tile([128, Q_TILE], F32R, tag="p")
                nc.scalar.activation(out=p, in_=st, func=AF.Exp, bias=shift_sb)
                pending.append((kt, p))
                if len(pending) > 2:
                    emit_pv(*pending.pop(0))
            for item in pending:
                emit_pv(*item)

            # ---- epilogue for this q_tile ----
            rd = epool.tile([1, Q_TILE], F32, tag="rd", bufs=1)
            nc.vector.reciprocal(out=rd, in_=dps)
            rd_dram = dpool.tile([1, Q_TILE], F32, tag="rd_dram")
            nc.sync.dma_start(out=rd_dram, in_=rd)
            rdb = epool.tile([128, Q_TILE], F32, tag="rdb", bufs=1)
            nc.sync.dma_start(out=rdb,
                              in_=rd_dram.to_broadcast([128, Q_TILE]))

            avs, a2s = [], []
            for c in range(CC):
                av = epool.tile([128, Q_TILE], F32, tag=f"av{c}",
                                name=f"av{c}", bufs=1)
                nc.scalar.copy(out=av, in_=acc_ap(acc1, c))
                a2 = epool.tile([128, Q_TILE], F32, tag=f"a2{c}",
                                name=f"a2{c}", bufs=1)
                nc.scalar.copy(out=a2, in_=acc_ap(acc2, c))
                avs.append(av)
                a2s.append(a2)

            for c in range(CC):
                # content ships uint8 (offset 127, scale folded into the
                # shipped mu/rstd); the cast recenters by -127.
                ctq = epool.tile([128, Q_TILE], mybir.dt.uint8, tag="ctq")
                nc.sync.dma_start(out=ctq, in_=ctr[c][:, qsl])
                ctf = epool.tile([128, Q_TILE], F32, tag="ctf", bufs=1)
                nc.scalar.activation(out=ctf, in_=ctq, func=AF.Copy,
                                     bias=-127.0)
                mean = avs[c]
                nc.vector.tensor_mul(mean, avs[c], rdb)
                e2 = a2s[c]
                nc.vector.tensor_mul(e2, a2s[c], rdb)
                var = epool.tile([128, Q_TILE], F32, tag="var", bufs=1)
                nc.vector.tensor_mul(var, mean, mean)
                nc.vector.scalar_tensor_tensor(
                    out=var, in0=var, scalar=-1.0, in1=e2,
                    op0=ALU.mult, op1=ALU.add)
                nc.vector.tensor_scalar_max(var, var, 1e-38)
                std = var
                nc.scalar.activation(out=std, in_=var, func=AF.Ln)
                nc.scalar.activation(out=std, in_=std, func=AF.Exp, scale=0.5)
                normc = epool.tile([128, Q_TILE], F32, tag="normc", bufs=1)
                nc.vector.tensor_scalar(
                    out=normc, in0=ctf,
                    scalar1=mu[:, c:c + 1], scalar2=rstd[:, c:c + 1],
                    op0=ALU.subtract, op1=ALU.mult)
                o = opool.tile([128, Q_TILE], F32, tag="o")
                nc.vector.tensor_mul(o, std, normc)
                o16 = opool.tile([128, Q_TILE], F16, tag="o16")
                nc.vector.tensor_add(o16, o, mean)
                om = epool.tile([128, 1], F32, tag="om", bufs=1)
                nc.vector.tensor_reduce(out=om, in_=o16,
                                        axis=mybir.AxisListType.X,
                                        op=ALU.max, apply_absolute_value=True)
                nc.vector.tensor_max(omax[:, c:c + 1], omax[:, c:c + 1], om)
                nc.sync.dma_start(out=scrr[c][:, qsl], in_=o16)

        # ---- pass 2: quantize the fp16 scratch to int8 with per-channel
        # scales (scale = max|out| / 127, computed above) ----
        qsc = persist.tile([128, CC], F32, tag="qsc")
        nc.vector.reciprocal(out=qsc, in_=omax)
        nc.vector.tensor_scalar_mul(qsc, qsc, 127.0)
        osc = persist.tile([128, CC], F32, tag="osc")
        nc.vector.tensor_scalar_mul(osc, omax, 1.0 / 127.0)
        nc.sync.dma_start(out=oscale[:], in_=osc)
        QW = 512
        for c in range(CC):
            for j in range(HW // QW):
                jsl = slice(j * QW, (j + 1) * QW)
                sof = opool.tile([128, QW], F16, tag="sof")
                nc.sync.dma_start(out=sof, in_=scrr[c][:, jsl])
                q8 = opool.tile([128, QW], mybir.dt.int8, tag="q8")
                nc.vector.tensor_scalar_mul(q8, sof, qsc[:, c:c + 1])
                nc.sync.dma_start(out=outr[c][:, jsl], in_=q8)

    # Force exp/ln/copy onto the shared natural_log_exp_and_others table
    # set: the default per-function choice alternates exp_and_others <->
    # natural_log, costing ~2.7us per ACT_TABLE_LOAD, dozens of times.
    import concourse.bacc as bacc_mod
    _orig_tables = bacc_mod.get_activation_tables
    _keep = "natural_log_exp_and_others"
    _strip = {AF.Exp, AF.Ln, AF.Copy, AF.Identity}

    def _patched_tables(arch):
        t = _orig_tables(arch)
        for name, fns in t.items():
            if name != _keep:
                t[name] = fns - _strip
        return t

    bacc_mod.get_activation_tables = _patched_tables
    try:
        nc.compile()
    finally:
        bacc_mod.get_activation_tables = _orig_tables
    return nc


_PROGRAM_CACHE = {}
_EXEC_CACHE = {}


def _get_program(key):
    if key not in _PROGRAM_CACHE:
        with_r, with_hb = key
        _PROGRAM_CACHE[key] = build_program(
            with_score_bias=with_r, with_v_bias=with_hb)
    return _PROGRAM_CACHE[key]


def prep_inputs(content, style, content_key, style_key, f_w, f_b, g_w, g_b,
                h_w, h_b):
    with_r = bool(np.any(np.asarray(f_b)))
    with_hb = bool(np.any(np.asarray(h_b)))
    key = (with_r, with_hb)

    ct32 = np.asarray(content, np.float32).reshape(B, C, HW)
    ck32 = np.asarray(content_key, np.float32).reshape(B, C, HW)
    sk32 = np.asarray(style_key, np.float32).reshape(B, C, HW)
    sty32 = np.asarray(style, np.float32).reshape(B, C, HW)

    f_w32 = np.asarray(f_w, np.float32)
    g_w32 = np.asarray(g_w, np.float32)
    wT16 = (g_w32.T @ f_w32).astype(np.float16)               # [C, C]
    hwT32 = np.ascontiguousarray(np.asarray(h_w, np.float32).T)
    u = (g_w32.T.astype(np.float64) @ np.asarray(f_b, np.float64)
         if with_r else None)
    hb16 = np.asarray(h_b, np.float16) if with_hb else None
    return key, (key, ct32, ck32, sk32, sty32, wT16, hwT32, u, hb16)


_F32_BUF = [None]


def _quant8u(x, out_u8):
    """Per-row 8-bit quantization of [C, N] into uint8 with offset 127.

    Writes floor(x/s + 127.5) into out_u8 (round-to-nearest of x/s,
    offset by 127 — the device cast recenters); returns the scale.
    Minimizes full-size passes: 2 reductions + multiply + add + cast.
    """
    s = np.maximum(np.maximum(x.max(axis=1), -x.min(axis=1)) / 127.0, 1e-30)
    if _F32_BUF[0] is None or _F32_BUF[0].shape != x.shape:
        _F32_BUF[0] = np.empty(x.shape, np.float32)
    buf = _F32_BUF[0]
    np.multiply(x, (1.0 / s)[:, None], out=buf)
    buf += np.float32(127.5)
    out_u8[...] = buf                      # f32 -> uint8 truncation = RTN
    return s


_BLOB_BUFS = {}


def make_blob(b, key, ct32, ck32, sk32, sty32, wT16, hwT32, u, hb16):
    with_r, with_hb = key
    offs, total = _blob_layout(with_r, with_hb)
    # Reuse per-batch buffers across calls: by the time kernel() returns,
    # every async put has been consumed (outputs were fetched), so the
    # buffers are free again.
    bufs = _BLOB_BUFS.setdefault(total, [None] * B)
    if bufs[b] is None:
        bufs[b] = np.empty(total, np.uint8)
    blob = bufs[b]

    def view(name, np_dt):
        o, n, isz = offs[name]
        return blob[o:o + n * isz].view(np_dt)

    view("ck", np.float16)[:] = ck32[b].reshape(-1)   # f32 -> f16 cast
    view("sk", np.float16)[:] = sk32[b].reshape(-1)
    view("wT", np.float16)[:] = wT16.reshape(-1)

    # style ships uint8; fold its per-channel scale into hwT's rows
    sty_s = _quant8u(sty32[b], view("sty", np.uint8).reshape(C, HW))
    view("hwT", np.float16)[:] = (hwT32 * sty_s[:, None]).reshape(-1)

    # content ships uint8; fold its scale into the shipped mu/rstd
    ctb = ct32[b]
    ct_s = _quant8u(ctb, view("ct", np.uint8).reshape(C, HW))
    mu = ctb.mean(axis=1)                                      # [C]
    m2 = np.einsum("ij,ij->i", ctb, ctb) / HW
    var = (m2 - mu * mu) * (HW / (HW - 1.0)) + 1e-5
    rstd = 1.0 / np.sqrt(var)
    mu_f = mu / ct_s                          # device computes (q-127-mu')*rstd'
    rstd_f = rstd * ct_s
    # stats layout [128, 2*CC]: col c = mu chunk c, col CC+c = rstd chunk
    st = view("stats", np.float16).reshape(128, 2 * CC)
    for c in range(CC):
        st[:, c] = mu_f[c * 128:(c + 1) * 128]
        st[:, CC + c] = rstd_f[c * 128:(c + 1) * 128]
    if with_r:
        view("rbias", np.float16)[:] = (
            u @ sk32[b].astype(np.float64)).astype(np.float16)
    if with_hb:
        view("hb", np.float16)[:] = hb16
    return blob


def make_in_maps(content, style, content_key, style_key, f_w, f_b, g_w, g_b,
                 h_w, h_b):
    key, prep = prep_inputs(content, style, content_key, style_key, f_w,
                            f_b, g_w, g_b, h_w, h_b)
    in_maps = [{"blob": make_blob(b, *prep)} for b in range(B)]
    return in_maps, key


def _get_exec(key):
    """Cached per-device jitted executors (one per core, no retrace).

    Per-core dispatch (instead of one shard_map call) lets the output
    fetch of core i overlap the input upload of core i+1 — the axon pipe
    is full-duplex, so the D2H leg hides entirely under the H2D stream.
    """
    if key in _EXEC_CACHE:
        return _EXEC_CACHE[key]
    import jax
    import jax.numpy as jnp
    from concourse.bass2jax import (
        _bass_exec_p, install_neuronx_cc_hook, partition_id_tensor)

    install_neuronx_cc_hook()
    nc = _get_program(key)
    assert nc.dbg_addr is None
    pname = nc.partition_id_tensor.name if nc.partition_id_tensor else None

    in_names, out_names, out_avals = [], [], []
    for alloc in nc.m.functions[0].allocations:
        if not isinstance(alloc, mybir.MemoryLocationSet):
            continue
        name = alloc.memorylocations[0].name
        if alloc.kind == "ExternalInput":
            if name != pname:
                in_names.append(name)
        elif alloc.kind == "ExternalOutput":
            out_names.append(name)
            out_avals.append(jax.core.ShapedArray(
                tuple(alloc.tensor_shape), mybir.dt.np(alloc.dtype)))
    assert in_names == ["blob"] and out_names == ["out", "oscale"]
    # Outputs are fully written by the kernel, so no pre-zeroed donated
    # output operands are needed (they'd cost an extra dispatch each call).
    all_in_names = list(in_names)
    if pname is not None:
        all_in_names.append(pname)

    def _body(*args):
        operands = list(args)
        if pname is not None:
            operands.append(partition_id_tensor())
        outs = _bass_exec_p.bind(
            *operands,
            out_avals=tuple(out_avals),
            in_names=tuple(all_in_names),
            out_names=tuple(out_names),
            lowering_input_output_aliases=(),
            sim_require_finite=True,
            sim_require_nnan=True,
            nc=nc,
        )
        return tuple(outs)

    devices = jax.devices()[:N_CORES]
    fns = [jax.jit(_body, keep_unused=True) for _ in devices]
    _EXEC_CACHE[key] = (fns, devices)
    return _EXEC_CACHE[key]


def kernel(**inputs):
    import jax
    key, prep = prep_inputs(**inputs)
    fns, devices = _get_exec(key)
    outs = []
    for b in range(B):
        blob = make_blob(b, *prep)             # host prep overlaps transfers
        xd = jax.device_put(blob, devices[b])
        o, osc = fns[b](xd)
        o.copy_to_host_async()
        osc.copy_to_host_async()
        outs.append((o, osc))
    res = np.empty((B, C, HW), np.float32)
    for b in range(B):
        o, osc = outs[b]
        q = np.asarray(o)                      # [C, HW] int8
        s = np.asarray(osc).T.reshape(C)       # [128, CC] -> per-channel
        np.multiply(q, s[:, None], out=res[b], casting="unsafe")
    return res.reshape(B, C, 64, 64)


if __name__ == "__main__":
    rng = np.random.default_rng(0)
    inputs = {
        "content": rng.standard_normal((B, C, 64, 64)).astype(np.float32),
        "style": rng.standard_normal((B, C, 64, 64)).astype(np.float32),
        "content_key": rng.standard_normal((B, C, 64, 64)).astype(np.float32),
        "style_key": rng.standard_normal((B, C, 64, 64)).astype(np.float32),
        "f_w": (rng.standard_normal((C, C)) * 0.02).astype(np.float32),
        "f_b": np.zeros(C, np.float32),
        "g_w": (rng.standard_normal((C, C)) * 0.02).astype(np.float32),
        "g_b": np.zeros(C, np.float32),
        "h_w": (rng.standard_normal((C, C)) * 0.02).astype(np.float32),
        "h_b": np.zeros(C, np.float32),
    }
    t0 = time.time()
    out = kernel(**inputs)
    print("kernel done", out.shape, out.dtype, time.time() - t0)
